# revision 46
# baseline (speedup 1.0000x reference)
# Trainium2 Bass kernel for nn_BDHBlock (dense transformer block).
#
# Strategy (8 NeuronCores, one shared SPMD program):
#   - Token-parallel for token-local stages: core c owns flat tokens
#     [512c, 512c+512). LayerNorm affine (g,b) is folded into the following
#     weights on the host, so on-chip LN is a pure normalize. All weights are
#     uploaded fp16 (pre-transposed, pre-shuffled), halving HBM traffic.
#   - Attention is head-parallel: core c owns global heads {2c, 2c+1} over the
#     full 4096-token sequence. q/k/v are exchanged with two AllToAlls split
#     by head parity (lo/hi) so the second exchange and the ctx return ride
#     under attention compute. Attention uses 512-wide q blocks with exact
#     causal masking; score->relu->ctx stages are decoupled across psum banks.
#   - fp16 matmuls everywhere (psum accumulate fp32); bias rows are seeded
#     into psum before accumulation chains to keep vector-engine work low.
import numpy as np

import concourse.bass as bass
import concourse.mybir as mybir
import concourse.tile as tile
from concourse import bacc
from concourse.masks import make_identity

B, S, H, NH = 2, 2048, 1024, 16
D = H // NH            # 64
FF = 4 * H             # 4096
NC = 8                 # cores
T = B * S // NC        # 512 tokens per core
TT = T // 128          # 4 token tiles
KT = H // 128          # 8 feature tiles
NFT = FF // 128        # 32 ffn tiles
F32, F16 = mybir.dt.float32, mybir.dt.float16
ADD, SUB, MUL, MAX = (mybir.AluOpType.add, mybir.AluOpType.subtract,
                      mybir.AluOpType.mult, mybir.AluOpType.max)
AF = mybir.ActivationFunctionType
RG = [list(range(NC))]
EPS = 1e-5

_CACHE = {}


def _build():
    nc = bacc.Bacc("TRN2", target_bir_lowering=False, debug=False,
                   num_devices=NC)

    # ---------------- I/O ----------------
    def inp(name, shape, dtype=F32):
        return nc.dram_tensor(name, list(shape), dtype, kind="ExternalInput")

    x_io = inp("x_c", (T, H))
    sfwT_io = inp("sfwT", (H, H), F16)
    wqT_io = inp("wqT", (H, H), F16)     # col-shuffled (parity, dest, d)
    wkT_io = inp("wkT", (H, H), F16)     # col-shuffled
    wvT_io = inp("wvT", (H, H), F16)     # col-shuffled
    woT_io = inp("woT", (H, H), F16)
    w1T_io = inp("w1T", (H, FF), F16)
    w2T_io = inp("w2T", (FF, H), F16)
    brow_io = inp("brow", (4, H), F16)   # rows: sb1, bv_shuf, bo, ff2_b
    bcol_io = inp("bcol", (128, 2 * KT))  # [p, nt]: bq_shuf, bk_shuf cols
    ff1b_col_io = inp("ff1b_col", (128, NFT))
    tri_io = inp("tri", (4, 128, 512))   # causal diag masks (f32)
    out_io = nc.dram_tensor("out_c", [T, H], F32, kind="ExternalOutput")

    # internal DRAM for collectives (HBM bounce)
    QSL = 64 * T                          # 32768: one head x 512 tokens
    a2a_in = [nc.dram_tensor(f"a2a{p}_in", [NC, 3, QSL], F16) for p in (0, 1)]
    a2a_out = [nc.dram_tensor(f"a2a{p}_out", [NC, 3, QSL], F16) for p in (0, 1)]
    cc_in = [nc.dram_tensor(f"cc{p}_in", [NC, QSL], F16) for p in (0, 1)]
    cc_out = [nc.dram_tensor(f"cc{p}_out", [NC, QSL], F16) for p in (0, 1)]
    warm_in = nc.dram_tensor("warm_in", [NC, 64], F16)
    warm_out = nc.dram_tensor("warm_out", [NC, 64], F16)

    from contextlib import ExitStack
    with tile.TileContext(nc) as tc, ExitStack() as es:
        # ---------------- pools ----------------
        const = es.enter_context(tc.tile_pool(name="const", bufs=1))
        persist = es.enter_context(tc.tile_pool(name="persist", bufs=1))
        wpool = es.enter_context(tc.tile_pool(name="wpool", bufs=6))
        att_pool = es.enter_context(tc.tile_pool(name="attp", bufs=18))
        sc_pool = es.enter_context(tc.tile_pool(name="scratch", bufs=2))
        small = es.enter_context(tc.tile_pool(name="small", bufs=4))
        rr_pool = es.enter_context(tc.tile_pool(name="rrp", bufs=2))
        pacc = es.enter_context(tc.tile_pool(name="pacc", bufs=1, space="PSUM"))
        pcx = es.enter_context(tc.tile_pool(name="pcx", bufs=2, space="PSUM"))
        pmix = es.enter_context(tc.tile_pool(name="pmix", bufs=2, space="PSUM"))

        ident16 = const.tile([128, 128], F16)
        make_identity(nc, ident16)
        # tiny warm-up AllToAll: absorbs launch skew + ring cold-start before
        # the first real exchange
        nc.gpsimd.dma_start(out=warm_in.ap(), in_=ident16[0:NC, 0:64])
        nc.gpsimd.collective_compute(
            "AllToAll", mybir.AluOpType.bypass, replica_groups=RG,
            ins=[warm_in.ap().opt()], outs=[warm_out.ap().opt()])
        tri = const.tile([128, 4, 512], F32)
        nc.sync.dma_start(out=tri[:], in_=tri_io.ap().rearrange("a p q -> p a q"))
        bcol = const.tile([128, 2 * KT], F32)
        nc.sync.dma_start(out=bcol[:], in_=bcol_io.ap())
        ff1b_col = const.tile([128, NFT], F32)
        nc.sync.dma_start(out=ff1b_col[:], in_=ff1b_col_io.ap())
        eps_col = const.tile([128, 1], F32)
        nc.vector.memset(eps_col[:], EPS)
        ones64 = const.tile([1, 64], F32)
        nc.vector.memset(ones64[:], 1.0)
        brow = const.tile([128, 4, H], F16)
        nc.sync.dma_start(
            out=brow[:], in_=brow_io.ap().unsqueeze(0).partition_broadcast(128))

        # residual stream, token-major [128, tt, H] (per-tile loads so LN1 can
        # start on tile 0 while the rest stream in)
        x_sb = persist.tile([128, TT, H], F32)
        for tt in range(TT):
            nc.sync.dma_start(
                out=x_sb[:, tt, :],
                in_=x_io.ap().rearrange("(tt p) h -> p tt h", p=128)[:, tt, :])

        lnT = persist.tile([128, KT, T], F16)       # LN output, feature-major
        kq_local = persist.tile([128, 2, 2, 4, T], F16, tag="bigA")  # [p,(k|q),par,m,t]
        vb_local = persist.tile([128, 2, NC, TT * 64], F16, tag="bigB")  # [p,par,j,(tt c)]
        qa = persist.tile([64, 2, B * S], F16, tag="bigC")  # [d, head, flat tok]
        ka = persist.tile([64, 2, B * S], F16, tag="bigD")
        va = persist.tile([128, 32, 2, D + 1], F16)  # [tokpart, fkt, head, d+1]
        ctxT = persist.tile([64, 2, B * S], F16)
        # slot-sharing: kq/vb are dead once the qkv A2As are staged; q/k are
        # dead once attention finishes -> reuse their SBUF for ctx and ffn h.
        ctxo_p = [persist.tile([64, KT, T], F16, tag="bigA", name="ctxo0"),
                  persist.tile([64, KT, T], F16, tag="bigB", name="ctxo1")]
        h_parts = [persist.tile([128, NFT // 2, T], F16, tag="bigC", name="h_lo"),
                   persist.tile([128, NFT // 2, T], F16, tag="bigD", name="h_hi")]
        stat4 = persist.tile([128, 6, TT], F32)     # sums/sq/mu/ex2/var/rstd
        junk = persist.tile([128, H], F32)

        nc.vector.memset(va[:], 1.0)                # bakes the ones column

        # ---------------- psum helpers ----------------
        _round = [0]

        def acc_tiles():
            r = _round[0]; _round[0] += 1
            if r % 2 == 0:
                return [pacc.tile([128, 512], F32, tag=f"acc{t}", name=f"acc{t}")
                        for t in range(4)]
            return [pcx.tile([128, 512], F32, tag="cx", name=f"apc{t}") for t in range(2)] + \
                   [pmix.tile([128, 512], F32, tag="pm", name=f"apm{t}") for t in range(2)]

        _eng = [0]

        def alt():
            _eng[0] ^= 1
            return nc.vector if _eng[0] else nc.scalar

        def copy_alt(out, in_):
            _eng[0] ^= 1
            if _eng[0]:
                nc.vector.tensor_copy(out, in_)
            else:
                nc.scalar.copy(out, in_)

        # ---------------- layernorm (pure normalize) -> lnT ----------------
        def layer_norm_t():
            for tt in range(TT):
                xt = x_sb[:, tt, :]
                sm = stat4[:, 0, tt:tt + 1]
                sq = stat4[:, 1, tt:tt + 1]
                mu = stat4[:, 2, tt:tt + 1]
                var = stat4[:, 4, tt:tt + 1]
                rstd = stat4[:, 5, tt:tt + 1]
                nc.vector.reduce_sum(sm, xt, axis=mybir.AxisListType.X)
                nc.scalar.activation(junk[:], xt, AF.Square, accum_out=sq)
                nc.vector.tensor_scalar_mul(mu, sm, 1.0 / H)
                nc.vector.tensor_scalar(var, mu, mu, -1.0, op0=MUL, op1=MUL)
                nc.vector.scalar_tensor_tensor(var, sq, 1.0 / H, var, op0=MUL, op1=ADD)
                nc.scalar.activation(rstd, var, AF.Sqrt, bias=eps_col[:])
                nc.vector.reciprocal(rstd, rstd)
                z16 = sc_pool.tile([128, H], F16, tag="z16")
                nc.vector.tensor_scalar(z16[:], xt, mu, rstd, op0=SUB, op1=MUL)
                for half in range(2):
                    pt = pmix.tile([128, 512], F16, tag="pm", name="pt")
                    for jj in range(4):
                        nc.tensor.transpose(pt[:, bass.ts(jj, 128)],
                                            z16[:, bass.ts(4 * half + jj, 128)], ident16[:])
                    copy_alt(lnT[:, 4 * half:4 * half + 4, bass.ts(tt, 128)],
                             pt[:].rearrange("p (a q) -> p a q", a=4))

        # ---------------- generic token-major linear + residual ----------------
        def linear_residual(wT_io_, brow_idx, src):
            """x += src.T @ w + bias ; src is lnT-style [128, KT, T] fp16."""
            psg = []
            for nch in range(2):
                ps = acc_tiles()
                psg.append(ps)
                for tt in range(TT):
                    nc.scalar.activation(ps[tt][:], brow[:, brow_idx, bass.ts(nch, 512)],
                                         AF.Copy)
                for kt in range(KT):
                    wt = wpool.tile([128, 512], F16, tag="w")
                    nc.sync.dma_start(out=wt[:], in_=wT_io_.ap()[bass.ts(kt, 128), bass.ts(nch, 512)])
                    for tt in range(TT):
                        nc.tensor.matmul(ps[tt][:], src[:, kt, bass.ts(tt, 128)],
                                         wt[:], start=False, stop=(kt == KT - 1),
                                         skip_group_check=True)
            for tt in range(TT):          # token-major so the next LN starts early
                for nch in range(2):
                    nc.vector.tensor_add(x_sb[:, tt, bass.ts(nch, 512)],
                                         x_sb[:, tt, bass.ts(nch, 512)], psg[nch][tt][:])

        # =====================================================================
        # Stage 1: x += LN1(x) @ (sf_w*mask*g1).T + (sfw@b1 + sf_b)
        # =====================================================================
        layer_norm_t()
        linear_residual(sfwT_io, 0, lnT)

        # =====================================================================
        # Stage 2: LN2 + QKV (parity-split) + A2A
        # =====================================================================
        layer_norm_t()

        def kq_proj(wio, which, par):
            """feature-major: kq_local[:, which, par, m, :] = w_chunk.T @ lnT + bias."""
            ps = acc_tiles()
            for kt in range(KT):
                wt = wpool.tile([128, 512], F16, tag="w")
                nc.sync.dma_start(out=wt[:], in_=wio.ap()[bass.ts(kt, 128), bass.ts(par, 512)])
                for m in range(4):
                    nc.tensor.matmul(ps[m][:], wt[:, bass.ts(m, 128)],
                                     lnT[:, kt, :],
                                     start=(kt == 0), stop=(kt == KT - 1))
            for m in range(4):
                nt = par * 4 + m
                bc = bcol[:, which * KT + nt: which * KT + nt + 1]
                e = alt()
                if e is nc.scalar:
                    nc.scalar.activation(kq_local[:, which, par, m, :], ps[m][:],
                                         AF.Identity, bias=bc)
                else:
                    nc.vector.tensor_scalar_add(kq_local[:, which, par, m, :], ps[m][:], bc)

        def v_proj(par):
            ps = acc_tiles()
            for tt in range(TT):
                nc.scalar.activation(ps[tt][:], brow[:, 1, bass.ts(par, 512)], AF.Copy)
            for kt in range(KT):
                wt = wpool.tile([128, 512], F16, tag="w")
                nc.sync.dma_start(out=wt[:], in_=wvT_io.ap()[bass.ts(kt, 128), bass.ts(par, 512)])
                for tt in range(TT):
                    nc.tensor.matmul(ps[tt][:], lnT[:, kt, bass.ts(tt, 128)],
                                     wt[:], start=False, stop=(kt == KT - 1),
                                     skip_group_check=True)
            for tt in range(TT):
                copy_alt(vb_local[:, par, :, bass.ds(64 * tt, 64)],
                         ps[tt][:].rearrange("p (j c) -> p j c", j=NC))

        def stage_qkv(par):
            # consolidated sends: slot j = 2m + half(partition)
            kq_view = a2a_in[par].ap().rearrange(
                "(m h) r (d t) -> h d r m t", h=2, d=64)
            for which in (0, 1):
                for half in (0, 1):
                    nc.gpsimd.dma_start(
                        out=kq_view[half, :, which, :, :],
                        in_=kq_local[bass.ts(half, 64), which, par, :, :])
            v_view = a2a_in[par].ap().rearrange(
                "j r (p x) -> p r j x", p=128)
            nc.gpsimd.dma_start(out=v_view[:, 2, :, :],
                                in_=vb_local[:, par, :, :])
            nc.gpsimd.collective_compute(
                "AllToAll", mybir.AluOpType.bypass, replica_groups=RG,
                ins=[a2a_in[par].ap().opt()], outs=[a2a_out[par].ap().opt()])

        def unstage_qkv(par):
            kq_view = a2a_out[par].ap().rearrange(
                "j r (d t) -> d r j t", d=64)
            nc.gpsimd.dma_start(
                out=ka[:, par, :].rearrange("p (j t) -> p j t", j=NC),
                in_=kq_view[:, 0, :, :])
            nc.gpsimd.dma_start(
                out=qa[:, par, :].rearrange("p (j t) -> p j t", j=NC),
                in_=kq_view[:, 1, :, :])
            v_view = a2a_out[par].ap().rearrange(
                "j r (p tt c) -> p r j tt c", p=128, tt=TT)
            for j in range(NC):
                nc.gpsimd.dma_start(
                    out=va[:, 4 * j:4 * j + 4, par, 0:D],
                    in_=v_view[:, 2, j, :, :])

        # parity A compute + send
        kq_proj(wkT_io, 0, 0)
        kq_proj(wqT_io, 1, 0)
        v_proj(0)
        stage_qkv(0)
        unstage_qkv(0)
        # parity B compute + send (overlaps A2A-A)
        kq_proj(wkT_io, 0, 1)
        kq_proj(wqT_io, 1, 1)
        v_proj(1)
        stage_qkv(1)
        unstage_qkv(1)

        # =====================================================================
        # Attention: heads {2c (par 0), 2c+1 (par 1)}, exact causal, relu-norm
        # =====================================================================
        def attention_head(h):
            # kt-outer / qp-inner: each k-tile's stationary (ka slice, va slice)
            # is loaded once and serves up to 4 back-to-back matmuls.  psum:
            # score(qp) -> pacc acc{qp}; cx[qp] -> pcx/pmix; rbp(qp) -> acc{qp}
            # (free after qp's last score).
            NQP = S // 512
            for b in range(B):
                base = b * S
                cxs = [pcx.tile([65, 512], F32, tag="cx", name=f"cx{qp}") for qp in (0, 1)] + \
                      [pmix.tile([65, 512], F32, tag="pm", name=f"cx{qp}") for qp in (2, 3)]
                alt_relu = [0]
                for kt in range(4 * NQP):
                    qp0 = kt // 4
                    scs = {}
                    for qp in range(qp0, NQP):
                        sc = pacc.tile([128, 512], F32, tag=f"acc{qp}", name="sc")
                        nc.tensor.matmul(sc[:], ka[:, h, bass.ds(base + kt * 128, 128)],
                                         qa[:, h, bass.ds(base + qp * 512, 512)],
                                         start=True, stop=True)
                        scs[qp] = sc
                    for qp in range(qp0, NQP):
                        sc = scs[qp]
                        att = att_pool.tile([128, 512], F16, tag="att")
                        if qp == qp0:
                            nc.vector.scalar_tensor_tensor(
                                att[:], sc[:], 0.0, tri[:, kt - 4 * qp, :],
                                op0=MAX, op1=MUL)
                        else:
                            nc.vector.tensor_relu(att[:], sc[:])
                        nc.tensor.matmul(cxs[qp][:], va[:, b * 16 + kt, h, :], att[:],
                                         start=(kt == 0), stop=(kt == 4 * qp + 3))
                    if kt % 4 == 3:
                        qp = qp0
                        cx = cxs[qp]
                        rs = rr_pool.tile([1, 512], F32, tag="rs")
                        rb = rr_pool.tile([64, 512], F32, tag="rb")
                        nc.vector.tensor_scalar_add(rs[:], cx[64:65, :], 1e-9)
                        nc.vector.reciprocal(rs[:], rs[:])
                        rbp = pacc.tile([64, 512], F32, tag=f"acc{qp}", name="rbp")
                        nc.tensor.matmul(rbp[:], ones64[:1, :], rs[:1, :],
                                         start=True, stop=True)
                        nc.scalar.copy(rb[:], rbp[:])
                        nc.vector.tensor_tensor(
                            ctxT[:, h, bass.ds(base + qp * 512, 512)],
                            cx[0:64, :], rb[:], MUL)

        def stage_ctx(par):
            nc.gpsimd.dma_start(
                out=cc_in[par].ap().rearrange("j (d t) -> d j t", d=64),
                in_=ctxT[:, par, :].rearrange("p (j t) -> p j t", j=NC))
            nc.gpsimd.collective_compute(
                "AllToAll", mybir.AluOpType.bypass, replica_groups=RG,
                ins=[cc_in[par].ap().opt()], outs=[cc_out[par].ap().opt()])

        def unstage_ctx(par):
            nc.gpsimd.dma_start(
                out=ctxo_p[par][:],
                in_=cc_out[par].ap().rearrange("j (d t) -> d j t", d=64))

        attention_head(0)
        stage_ctx(0)
        attention_head(1)
        unstage_ctx(0)
        stage_ctx(1)
        unstage_ctx(1)

        # =====================================================================
        # out-proj: x += ctx @ wo.T + bo, parity-split so the par-0 half of the
        # contraction runs while the parity-1 ctx AllToAll is still in flight
        # =====================================================================
        psg = [acc_tiles(), acc_tiles()]
        for nch in range(2):
            for tt in range(TT):
                nc.scalar.activation(psg[nch][tt][:], brow[:, 2, bass.ts(nch, 512)],
                                     AF.Copy)
        for par in range(2):
            for nch in range(2):
                for kt in range(KT):
                    wt = wpool.tile([64, 512], F16, tag="wh")
                    nc.sync.dma_start(
                        out=wt[:],
                        in_=woT_io.ap()[bass.ds(128 * kt + 64 * par, 64), bass.ts(nch, 512)])
                    for tt in range(TT):
                        nc.tensor.matmul(psg[nch][tt][:],
                                         ctxo_p[par][:, kt, bass.ts(tt, 128)],
                                         wt[:], start=False,
                                         stop=(par == 1 and kt == KT - 1),
                                         skip_group_check=True)
        for tt in range(TT):
            for nch in range(2):
                nc.vector.tensor_add(x_sb[:, tt, bass.ts(nch, 512)],
                                     x_sb[:, tt, bass.ts(nch, 512)], psg[nch][tt][:])

        # =====================================================================
        # FFN: x += relu(LN3(x) @ w1'.T + b1f) @ w2.T + ff2_b
        # =====================================================================
        layer_norm_t()
        for nh in range(NFT // 4):
            ps = acc_tiles()
            for kt in range(KT):
                wt = wpool.tile([128, 512], F16, tag="w")
                nc.sync.dma_start(out=wt[:], in_=w1T_io.ap()[bass.ts(kt, 128), bass.ts(nh, 512)])
                for m in range(4):
                    nc.tensor.matmul(ps[m][:], wt[:, bass.ts(m, 128)],
                                     lnT[:, kt, :],
                                     start=(kt == 0), stop=(kt == KT - 1))
            for m in range(4):
                nt = nh * 4 + m
                hd = h_parts[nt // (NFT // 2)][:, nt % (NFT // 2), :]
                if m % 2 == 0:
                    nc.scalar.activation(hd, ps[m][:], AF.Relu,
                                         bias=ff1b_col[:, nt:nt + 1])
                else:
                    nc.vector.tensor_scalar(hd, ps[m][:],
                                            ff1b_col[:, nt:nt + 1], 0.0,
                                            op0=ADD, op1=MAX)
        for nch in range(2):
            ps = acc_tiles()
            for tt in range(TT):
                nc.scalar.activation(ps[tt][:], brow[:, 3, bass.ts(nch, 512)], AF.Copy)
            for kt in range(NFT):
                wt = wpool.tile([128, 512], F16, tag="w")
                nc.sync.dma_start(out=wt[:], in_=w2T_io.ap()[bass.ts(kt, 128), bass.ts(nch, 512)])
                hsrc = h_parts[kt // (NFT // 2)]
                for tt in range(TT):
                    nc.tensor.matmul(ps[tt][:], hsrc[:, kt % (NFT // 2), bass.ts(tt, 128)],
                                     wt[:], start=False, stop=(kt == NFT - 1),
                                     skip_group_check=True)
            for tt in range(TT):
                nc.vector.tensor_add(x_sb[:, tt, bass.ts(nch, 512)],
                                     x_sb[:, tt, bass.ts(nch, 512)], ps[tt][:])
            # stream this half of the output while the other half finishes
            nc.sync.dma_start(
                out=out_io.ap().rearrange("(tt p) h -> p tt h", p=128)[:, :, bass.ts(nch, 512)],
                in_=x_sb[:, :, bass.ts(nch, 512)])

    nc.compile()
    return nc


def _prep_shared(inputs):
    f = lambda a: np.asarray(a, np.float32)
    f16 = lambda a: np.ascontiguousarray(np.asarray(a, np.float16))
    g1, b1 = f(inputs["g1"]), f(inputs["b1"])
    g2, b2 = f(inputs["g2"]), f(inputs["b2"])
    g3, b3 = f(inputs["g3"]), f(inputs["b3"])

    # stage-1 sparse linear with LN1 affine folded in
    wsf = f(inputs["sf_w"]) * f(inputs["mask"])
    sb1 = wsf @ b1 + f(inputs["sf_b"])
    wsf = wsf * g1[None, :]

    # qkv with LN2 affine folded; q/k pre-scaled by D**-0.25 each
    qsc = float(D) ** -0.25
    wq = f(inputs["wq"]); bq = (wq @ b2 + f(inputs["bq"])) * qsc
    wq = wq * g2[None, :] * qsc
    wk = f(inputs["wk"]); bk = (wk @ b2 + f(inputs["bk"])) * qsc
    wk = wk * g2[None, :] * qsc
    wv = f(inputs["wv"]); bv = wv @ b2 + f(inputs["bv"])
    wv = wv * g2[None, :]

    # ffn with LN3 affine folded
    w1 = f(inputs["ff1_w"]); b1f = w1 @ b3 + f(inputs["ff1_b"])
    w1 = w1 * g3[None, :]

    # column shuffle for head-parity A2A: s = par*512 + j*64 + c  <-  128j+64par+c
    perm = np.empty(H, np.int64)
    for par in range(2):
        for j in range(NC):
            s0 = par * 512 + j * 64
            perm[s0:s0 + 64] = 128 * j + 64 * par + np.arange(64)

    sh = {
        "sfwT": f16(wsf.T),
        "wqT": f16(wq.T[:, perm]),
        "wkT": f16(wk.T[:, perm]),
        "wvT": f16(wv.T[:, perm]),
        "woT": f16(f(inputs["wo"]).T),
        "w1T": f16(w1.T),
        "w2T": f16(f(inputs["ff2_w"]).T),
        "brow": np.ascontiguousarray(np.stack(
            [sb1, bv[perm], f(inputs["bo"]), f(inputs["ff2_b"])]).astype(np.float16)),
        "bcol": np.ascontiguousarray(
            np.stack([bk[perm], bq[perm]]).reshape(2 * KT, 128).T.copy().astype(np.float32)),
        "ff1b_col": np.ascontiguousarray(b1f.reshape(NFT, 128).T.copy().astype(np.float32)),
    }
    # diag masks: tri[i][p, c] = 1 if 128*i + p <= c else 0 (c in 0..512)
    tri = np.zeros((4, 128, 512), np.float32)
    for i in range(4):
        p = np.arange(128)[:, None] + 128 * i
        c = np.arange(512)[None, :]
        tri[i] = (p <= c).astype(np.float32)
    sh["tri"] = tri
    return sh


def kernel(**inputs) -> np.ndarray:
    from concourse.bass_utils import run_bass_kernel_spmd

    if "nc" not in _CACHE:
        _CACHE["nc"] = _build()
    nc = _CACHE["nc"]

    sh = _prep_shared(inputs)
    x = np.ascontiguousarray(np.asarray(inputs["x"], np.float32)).reshape(B * S, H)
    in_maps = []
    for c in range(NC):
        m = dict(sh)
        m["x_c"] = np.ascontiguousarray(x[c * T:(c + 1) * T])
        in_maps.append(m)

    res = run_bass_kernel_spmd(nc, in_maps, core_ids=list(range(NC)))
    out = np.concatenate([res.results[c]["out_c"] for c in range(NC)], axis=0)
    return out.reshape(B, S, H).astype(np.float32)


# revision 50
# speedup vs baseline: 1.1064x; 1.1064x over previous
# Trainium2 Bass kernel for nn_BDHBlock (dense transformer block).
#
# Strategy (8 NeuronCores, one shared SPMD program):
#   - Token-parallel for token-local stages: core c owns flat tokens
#     [512c, 512c+512). LayerNorm affine (g,b) is folded into the following
#     weights on the host, so on-chip LN is a pure normalize. All weights are
#     uploaded fp16 (pre-transposed, pre-shuffled), halving HBM traffic.
#   - Attention is head-parallel: core c owns global heads {2c, 2c+1} over the
#     full 4096-token sequence. q/k/v are exchanged with two AllToAlls split
#     by head parity (lo/hi) so the second exchange and the ctx return ride
#     under attention compute. Attention uses 512-wide q blocks with exact
#     causal masking; score->relu->ctx stages are decoupled across psum banks.
#   - fp16 matmuls everywhere (psum accumulate fp32); bias rows are seeded
#     into psum before accumulation chains to keep vector-engine work low.
import numpy as np

import concourse.bass as bass
import concourse.mybir as mybir
import concourse.tile as tile
from concourse import bacc
from concourse.masks import make_identity

B, S, H, NH = 2, 2048, 1024, 16
D = H // NH            # 64
FF = 4 * H             # 4096
NC = 8                 # cores
T = B * S // NC        # 512 tokens per core
TT = T // 128          # 4 token tiles
KT = H // 128          # 8 feature tiles
NFT = FF // 128        # 32 ffn tiles
F32, F16 = mybir.dt.float32, mybir.dt.float16
ADD, SUB, MUL, MAX = (mybir.AluOpType.add, mybir.AluOpType.subtract,
                      mybir.AluOpType.mult, mybir.AluOpType.max)
AF = mybir.ActivationFunctionType
RG = [list(range(NC))]
EPS = 1e-5

_CACHE = {}


def _build():
    nc = bacc.Bacc("TRN2", target_bir_lowering=False, debug=False,
                   num_devices=NC)

    # ---------------- I/O ----------------
    def inp(name, shape, dtype=F32):
        return nc.dram_tensor(name, list(shape), dtype, kind="ExternalInput")

    x_io = inp("x_c", (T, H))
    sfwT_io = inp("sfwT", (H, H), F16)
    wqT_io = inp("wqT", (H, H), F16)     # col-shuffled (parity, dest, d)
    wkT_io = inp("wkT", (H, H), F16)     # col-shuffled
    wvT_io = inp("wvT", (H, H), F16)     # col-shuffled
    woT_io = inp("woT", (H, H), F16)
    w1T_io = inp("w1T", (H, FF), F16)
    w2T_io = inp("w2T", (FF, H), F16)
    brow_io = inp("brow", (4, H), F16)   # rows: sb1, bv_shuf, bo, ff2_b
    bcol_io = inp("bcol", (128, 2 * KT))  # [p, nt]: bq_shuf, bk_shuf cols
    ff1b_col_io = inp("ff1b_col", (128, NFT))
    tri_io = inp("tri", (4, 128, 512))   # causal diag masks (f32)
    out_io = nc.dram_tensor("out_c", [T, H], F32, kind="ExternalOutput")

    # internal DRAM for collectives (HBM bounce)
    QSL = 64 * T                          # 32768: one head x 512 tokens
    a2a_in = [nc.dram_tensor(f"a2a{p}_in", [NC, 3, QSL], F16) for p in (0, 1)]
    a2a_out = [nc.dram_tensor(f"a2a{p}_out", [NC, 3, QSL], F16) for p in (0, 1)]
    cc_in = [nc.dram_tensor(f"cc{p}_in", [NC, QSL], F16) for p in (0, 1)]
    cc_out = [nc.dram_tensor(f"cc{p}_out", [NC, QSL], F16) for p in (0, 1)]
    warm_in = nc.dram_tensor("warm_in", [NC, 64], F16)
    warm_out = nc.dram_tensor("warm_out", [NC, 64], F16)

    from contextlib import ExitStack
    with tile.TileContext(nc) as tc, ExitStack() as es:
        # ---------------- pools ----------------
        const = es.enter_context(tc.tile_pool(name="const", bufs=1))
        persist = es.enter_context(tc.tile_pool(name="persist", bufs=1))
        wpool = es.enter_context(tc.tile_pool(name="wpool", bufs=6))
        att_pool = es.enter_context(tc.tile_pool(name="attp", bufs=18))
        sc_pool = es.enter_context(tc.tile_pool(name="scratch", bufs=2))
        small = es.enter_context(tc.tile_pool(name="small", bufs=4))
        rr_pool = es.enter_context(tc.tile_pool(name="rrp", bufs=2))
        pacc = es.enter_context(tc.tile_pool(name="pacc", bufs=1, space="PSUM"))
        pcx = es.enter_context(tc.tile_pool(name="pcx", bufs=2, space="PSUM"))
        pmix = es.enter_context(tc.tile_pool(name="pmix", bufs=2, space="PSUM"))

        ident16 = const.tile([128, 128], F16)
        make_identity(nc, ident16)
        # tiny warm-up AllToAll: absorbs launch skew + ring cold-start before
        # the first real exchange
        nc.gpsimd.dma_start(out=warm_in.ap(), in_=ident16[0:NC, 0:64])
        nc.gpsimd.collective_compute(
            "AllToAll", mybir.AluOpType.bypass, replica_groups=RG,
            ins=[warm_in.ap().opt()], outs=[warm_out.ap().opt()])
        tri = const.tile([128, 4, 512], F32)
        nc.sync.dma_start(out=tri[:], in_=tri_io.ap().rearrange("a p q -> p a q"))
        bcol = const.tile([128, 2 * KT], F32)
        nc.sync.dma_start(out=bcol[:], in_=bcol_io.ap())
        ff1b_col = const.tile([128, NFT], F32)
        nc.sync.dma_start(out=ff1b_col[:], in_=ff1b_col_io.ap())
        eps_col = const.tile([128, 1], F32)
        nc.vector.memset(eps_col[:], EPS)
        ones64 = const.tile([1, 64], F32)
        nc.vector.memset(ones64[:], 1.0)
        brow = const.tile([128, 4, H], F16)
        nc.sync.dma_start(
            out=brow[:], in_=brow_io.ap().unsqueeze(0).partition_broadcast(128))

        # residual stream, token-major [128, tt, H] (per-tile loads so LN1 can
        # start on tile 0 while the rest stream in)
        x_sb = persist.tile([128, TT, H], F32)
        for tt in range(TT):
            nc.sync.dma_start(
                out=x_sb[:, tt, :],
                in_=x_io.ap().rearrange("(tt p) h -> p tt h", p=128)[:, tt, :])

        lnT = persist.tile([128, KT, T], F16)       # LN output, feature-major
        kq_local = persist.tile([128, 2, 2, 4, T], F16, tag="bigA")  # [p,(k|q),par,m,t]
        vb_local = persist.tile([128, 2, NC, TT * 64], F16, tag="bigB")  # [p,par,j,(tt c)]
        qa = persist.tile([64, 2, B * S], F16, tag="bigC")  # [d, head, flat tok]
        ka = persist.tile([64, 2, B * S], F16, tag="bigD")
        va = persist.tile([128, 32, 2, D + 1], F16)  # [tokpart, fkt, head, d+1]
        va64 = persist.tile([128, 2, 32, D], F16)    # contiguous A2A landing pad
        ctxT = persist.tile([64, 2, B * S], F16)
        # slot-sharing: kq/vb are dead once the qkv A2As are staged; q/k are
        # dead once attention finishes -> reuse their SBUF for ctx and ffn h.
        ctxo_p = [persist.tile([64, KT, T], F16, tag="bigA", name="ctxo0"),
                  persist.tile([64, KT, T], F16, tag="bigB", name="ctxo1")]
        h_parts = [persist.tile([128, NFT // 2, T], F16, tag="bigC", name="h_lo"),
                   persist.tile([128, NFT // 2, T], F16, tag="bigD", name="h_hi")]
        stat4 = persist.tile([128, 6, TT], F32)     # sums/sq/mu/ex2/var/rstd
        junk = persist.tile([128, H], F32)

        nc.vector.memset(va[:], 1.0)                # bakes the ones column

        # ---------------- psum helpers ----------------
        _round = [0]

        def acc_tiles():
            r = _round[0]; _round[0] += 1
            if r % 2 == 0:
                return [pacc.tile([128, 512], F32, tag=f"acc{t}", name=f"acc{t}")
                        for t in range(4)]
            return [pcx.tile([128, 512], F32, tag="cx", name=f"apc{t}") for t in range(2)] + \
                   [pmix.tile([128, 512], F32, tag="pm", name=f"apm{t}") for t in range(2)]

        _eng = [0]

        def alt():
            _eng[0] ^= 1
            return nc.vector if _eng[0] else nc.scalar

        def copy_alt(out, in_):
            _eng[0] ^= 1
            if _eng[0]:
                nc.vector.tensor_copy(out, in_)
            else:
                nc.scalar.copy(out, in_)

        # ---------------- layernorm (pure normalize) -> lnT ----------------
        def layer_norm_t():
            for tt in range(TT):
                xt = x_sb[:, tt, :]
                sm = stat4[:, 0, tt:tt + 1]
                sq = stat4[:, 1, tt:tt + 1]
                mu = stat4[:, 2, tt:tt + 1]
                var = stat4[:, 4, tt:tt + 1]
                rstd = stat4[:, 5, tt:tt + 1]
                nc.vector.reduce_sum(sm, xt, axis=mybir.AxisListType.X)
                nc.scalar.activation(junk[:], xt, AF.Square, accum_out=sq)
                nc.vector.tensor_scalar_mul(mu, sm, 1.0 / H)
                nc.vector.tensor_scalar(var, mu, mu, -1.0, op0=MUL, op1=MUL)
                nc.vector.scalar_tensor_tensor(var, sq, 1.0 / H, var, op0=MUL, op1=ADD)
                nc.scalar.activation(rstd, var, AF.Sqrt, bias=eps_col[:])
                nc.vector.reciprocal(rstd, rstd)
                z16 = sc_pool.tile([128, H], F16, tag="z16")
                nc.vector.tensor_scalar(z16[:], xt, mu, rstd, op0=SUB, op1=MUL)
                for half in range(2):
                    pt = pmix.tile([128, 512], F16, tag="pm", name="pt")
                    for jj in range(4):
                        nc.tensor.transpose(pt[:, bass.ts(jj, 128)],
                                            z16[:, bass.ts(4 * half + jj, 128)], ident16[:])
                    copy_alt(lnT[:, 4 * half:4 * half + 4, bass.ts(tt, 128)],
                             pt[:].rearrange("p (a q) -> p a q", a=4))

        # ---------------- generic token-major linear + residual ----------------
        def linear_residual(wT_io_, brow_idx, src):
            """x += src.T @ w + bias ; src is lnT-style [128, KT, T] fp16."""
            psg = []
            for nch in range(2):
                ps = acc_tiles()
                psg.append(ps)
                for tt in range(TT):
                    nc.scalar.activation(ps[tt][:], brow[:, brow_idx, bass.ts(nch, 512)],
                                         AF.Copy)
                for kt in range(KT):
                    wt = wpool.tile([128, 512], F16, tag="w")
                    nc.sync.dma_start(out=wt[:], in_=wT_io_.ap()[bass.ts(kt, 128), bass.ts(nch, 512)])
                    for tt in range(TT):
                        nc.tensor.matmul(ps[tt][:], src[:, kt, bass.ts(tt, 128)],
                                         wt[:], start=False, stop=(kt == KT - 1),
                                         skip_group_check=True)
            for tt in range(TT):          # token-major so the next LN starts early
                for nch in range(2):
                    nc.vector.tensor_add(x_sb[:, tt, bass.ts(nch, 512)],
                                         x_sb[:, tt, bass.ts(nch, 512)], psg[nch][tt][:])

        # =====================================================================
        # Stage 1: x += LN1(x) @ (sf_w*mask*g1).T + (sfw@b1 + sf_b)
        # =====================================================================
        layer_norm_t()
        linear_residual(sfwT_io, 0, lnT)

        # =====================================================================
        # Stage 2: LN2 + QKV (parity-split) + A2A
        # =====================================================================
        layer_norm_t()

        def kq_proj(wio, which, par):
            """feature-major: kq_local[:, which, par, m, :] = w_chunk.T @ lnT + bias."""
            ps = acc_tiles()
            for kt in range(KT):
                wt = wpool.tile([128, 512], F16, tag="w")
                nc.sync.dma_start(out=wt[:], in_=wio.ap()[bass.ts(kt, 128), bass.ts(par, 512)])
                for m in range(4):
                    nc.tensor.matmul(ps[m][:], wt[:, bass.ts(m, 128)],
                                     lnT[:, kt, :],
                                     start=(kt == 0), stop=(kt == KT - 1))
            for m in range(4):
                nt = par * 4 + m
                bc = bcol[:, which * KT + nt: which * KT + nt + 1]
                e = alt()
                if e is nc.scalar:
                    nc.scalar.activation(kq_local[:, which, par, m, :], ps[m][:],
                                         AF.Identity, bias=bc)
                else:
                    nc.vector.tensor_scalar_add(kq_local[:, which, par, m, :], ps[m][:], bc)

        def v_proj(par):
            ps = acc_tiles()
            for tt in range(TT):
                nc.scalar.activation(ps[tt][:], brow[:, 1, bass.ts(par, 512)], AF.Copy)
            for kt in range(KT):
                wt = wpool.tile([128, 512], F16, tag="w")
                nc.sync.dma_start(out=wt[:], in_=wvT_io.ap()[bass.ts(kt, 128), bass.ts(par, 512)])
                for tt in range(TT):
                    nc.tensor.matmul(ps[tt][:], lnT[:, kt, bass.ts(tt, 128)],
                                     wt[:], start=False, stop=(kt == KT - 1),
                                     skip_group_check=True)
            for tt in range(TT):
                copy_alt(vb_local[:, par, :, bass.ds(64 * tt, 64)],
                         ps[tt][:].rearrange("p (j c) -> p j c", j=NC))

        def stage_qkv(par):
            # consolidated sends: slot j = 2m + half(partition)
            kq_view = a2a_in[par].ap().rearrange(
                "(m h) r (d t) -> h d r m t", h=2, d=64)
            for which in (0, 1):
                for half in (0, 1):
                    nc.gpsimd.dma_start(
                        out=kq_view[half, :, which, :, :],
                        in_=kq_local[bass.ts(half, 64), which, par, :, :])
            v_view = a2a_in[par].ap().rearrange(
                "j r (p x) -> p r j x", p=128)
            nc.gpsimd.dma_start(out=v_view[:, 2, :, :],
                                in_=vb_local[:, par, :, :])
            nc.gpsimd.collective_compute(
                "AllToAll", mybir.AluOpType.bypass, replica_groups=RG,
                ins=[a2a_in[par].ap().opt()], outs=[a2a_out[par].ap().opt()])

        def unstage_qkv(par):
            kq_view = a2a_out[par].ap().rearrange(
                "j r (d t) -> d r j t", d=64)
            nc.gpsimd.dma_start(
                out=ka[:, par, :].rearrange("p (j t) -> p j t", j=NC),
                in_=kq_view[:, 0, :, :])
            nc.gpsimd.dma_start(
                out=qa[:, par, :].rearrange("p (j t) -> p j t", j=NC),
                in_=kq_view[:, 1, :, :])
            v_view = a2a_out[par].ap().rearrange(
                "j r (p x) -> p r j x", p=128)
            nc.gpsimd.dma_start(
                out=va64[:, par, :, :].rearrange("p (j tt) c -> p j (tt c)", j=NC),
                in_=v_view[:, 2, :, :])
            nc.gpsimd.tensor_copy(va[:, :, par, 0:D], va64[:, par, :, :])

        # parity A compute + send
        kq_proj(wkT_io, 0, 0)
        kq_proj(wqT_io, 1, 0)
        v_proj(0)
        stage_qkv(0)
        unstage_qkv(0)
        # parity B compute + send (overlaps A2A-A)
        kq_proj(wkT_io, 0, 1)
        kq_proj(wqT_io, 1, 1)
        v_proj(1)
        stage_qkv(1)
        unstage_qkv(1)

        # =====================================================================
        # Attention: heads {2c (par 0), 2c+1 (par 1)}, exact causal, relu-norm
        # =====================================================================
        def attention_head(h):
            # kt-outer / qp-inner: each k-tile's stationary (ka slice, va slice)
            # is loaded once and serves up to 4 back-to-back matmuls.  psum:
            # score(qp) -> pacc acc{qp}; cx[qp] -> pcx/pmix; rbp(qp) -> acc{qp}
            # (free after qp's last score).
            NQP = S // 512
            for b in range(B):
                base = b * S
                cxs = [pcx.tile([65, 512], F32, tag="cx", name=f"cx{qp}") for qp in (0, 1)] + \
                      [pmix.tile([65, 512], F32, tag="pm", name=f"cx{qp}") for qp in (2, 3)]
                alt_relu = [0]
                for kt in range(4 * NQP):
                    qp0 = kt // 4
                    scs = {}
                    for qp in range(qp0, NQP):
                        sc = pacc.tile([128, 512], F32, tag=f"acc{qp}", name="sc")
                        nc.tensor.matmul(sc[:], ka[:, h, bass.ds(base + kt * 128, 128)],
                                         qa[:, h, bass.ds(base + qp * 512, 512)],
                                         start=True, stop=True)
                        scs[qp] = sc
                    for qp in range(qp0, NQP):
                        sc = scs[qp]
                        att = att_pool.tile([128, 512], F16, tag="att")
                        if qp == qp0:
                            nc.vector.scalar_tensor_tensor(
                                att[:], sc[:], 0.0, tri[:, kt - 4 * qp, :],
                                op0=MAX, op1=MUL)
                        else:
                            alt_relu[0] ^= 1
                            if alt_relu[0]:
                                nc.scalar.activation(att[:], sc[:], AF.Relu)
                            else:
                                nc.vector.tensor_relu(att[:], sc[:])
                        nc.tensor.matmul(cxs[qp][:], va[:, b * 16 + kt, h, :], att[:],
                                         start=(kt == 0), stop=(kt == 4 * qp + 3))
                    if kt % 4 == 3:
                        qp = qp0
                        cx = cxs[qp]
                        rs = rr_pool.tile([1, 512], F32, tag="rs")
                        rb = rr_pool.tile([64, 512], F32, tag="rb")
                        nc.vector.tensor_scalar_add(rs[:], cx[64:65, :], 1e-9)
                        nc.vector.reciprocal(rs[:], rs[:])
                        rbp = pacc.tile([64, 512], F32, tag=f"acc{qp}", name="rbp")
                        nc.tensor.matmul(rbp[:], ones64[:1, :], rs[:1, :],
                                         start=True, stop=True)
                        nc.scalar.copy(rb[:], rbp[:])
                        nc.vector.tensor_tensor(
                            ctxT[:, h, bass.ds(base + qp * 512, 512)],
                            cx[0:64, :], rb[:], MUL)

        def stage_ctx(par):
            nc.gpsimd.dma_start(
                out=cc_in[par].ap().rearrange("j (d t) -> d j t", d=64),
                in_=ctxT[:, par, :].rearrange("p (j t) -> p j t", j=NC))
            nc.gpsimd.collective_compute(
                "AllToAll", mybir.AluOpType.bypass, replica_groups=RG,
                ins=[cc_in[par].ap().opt()], outs=[cc_out[par].ap().opt()])

        def unstage_ctx(par):
            nc.gpsimd.dma_start(
                out=ctxo_p[par][:],
                in_=cc_out[par].ap().rearrange("j (d t) -> d j t", d=64))

        attention_head(0)
        stage_ctx(0)
        attention_head(1)
        unstage_ctx(0)
        stage_ctx(1)
        unstage_ctx(1)

        # =====================================================================
        # out-proj: x += ctx @ wo.T + bo, parity-split so the par-0 half of the
        # contraction runs while the parity-1 ctx AllToAll is still in flight
        # =====================================================================
        psg = [acc_tiles(), acc_tiles()]
        for nch in range(2):
            for tt in range(TT):
                nc.scalar.activation(psg[nch][tt][:], brow[:, 2, bass.ts(nch, 512)],
                                     AF.Copy)
        for par in range(2):
            for nch in range(2):
                for kt in range(KT):
                    wt = wpool.tile([64, 512], F16, tag="wh")
                    nc.sync.dma_start(
                        out=wt[:],
                        in_=woT_io.ap()[bass.ds(128 * kt + 64 * par, 64), bass.ts(nch, 512)])
                    for tt in range(TT):
                        nc.tensor.matmul(psg[nch][tt][:],
                                         ctxo_p[par][:, kt, bass.ts(tt, 128)],
                                         wt[:], start=False,
                                         stop=(par == 1 and kt == KT - 1),
                                         skip_group_check=True)
        for tt in range(TT):
            for nch in range(2):
                nc.vector.tensor_add(x_sb[:, tt, bass.ts(nch, 512)],
                                     x_sb[:, tt, bass.ts(nch, 512)], psg[nch][tt][:])

        # =====================================================================
        # FFN: x += relu(LN3(x) @ w1'.T + b1f) @ w2.T + ff2_b
        # =====================================================================
        layer_norm_t()
        for nh in range(NFT // 4):
            ps = acc_tiles()
            for kt in range(KT):
                wt = wpool.tile([128, 512], F16, tag="w")
                nc.sync.dma_start(out=wt[:], in_=w1T_io.ap()[bass.ts(kt, 128), bass.ts(nh, 512)])
                for m in range(4):
                    nc.tensor.matmul(ps[m][:], wt[:, bass.ts(m, 128)],
                                     lnT[:, kt, :],
                                     start=(kt == 0), stop=(kt == KT - 1))
            for m in range(4):
                nt = nh * 4 + m
                hd = h_parts[nt // (NFT // 2)][:, nt % (NFT // 2), :]
                if m % 2 == 0:
                    nc.scalar.activation(hd, ps[m][:], AF.Relu,
                                         bias=ff1b_col[:, nt:nt + 1])
                else:
                    nc.vector.tensor_scalar(hd, ps[m][:],
                                            ff1b_col[:, nt:nt + 1], 0.0,
                                            op0=ADD, op1=MAX)
        for nch in range(2):
            ps = acc_tiles()
            for tt in range(TT):
                nc.scalar.activation(ps[tt][:], brow[:, 3, bass.ts(nch, 512)], AF.Copy)
            for kt in range(NFT):
                wt = wpool.tile([128, 512], F16, tag="w")
                nc.sync.dma_start(out=wt[:], in_=w2T_io.ap()[bass.ts(kt, 128), bass.ts(nch, 512)])
                hsrc = h_parts[kt // (NFT // 2)]
                for tt in range(TT):
                    nc.tensor.matmul(ps[tt][:], hsrc[:, kt % (NFT // 2), bass.ts(tt, 128)],
                                     wt[:], start=False, stop=(kt == NFT - 1),
                                     skip_group_check=True)
            for tt in range(TT):
                nc.vector.tensor_add(x_sb[:, tt, bass.ts(nch, 512)],
                                     x_sb[:, tt, bass.ts(nch, 512)], ps[tt][:])
            # stream this half of the output while the other half finishes
            nc.sync.dma_start(
                out=out_io.ap().rearrange("(tt p) h -> p tt h", p=128)[:, :, bass.ts(nch, 512)],
                in_=x_sb[:, :, bass.ts(nch, 512)])

    nc.compile()
    return nc


def _prep_shared(inputs):
    f = lambda a: np.asarray(a, np.float32)
    f16 = lambda a: np.ascontiguousarray(np.asarray(a, np.float16))
    g1, b1 = f(inputs["g1"]), f(inputs["b1"])
    g2, b2 = f(inputs["g2"]), f(inputs["b2"])
    g3, b3 = f(inputs["g3"]), f(inputs["b3"])

    # stage-1 sparse linear with LN1 affine folded in
    wsf = f(inputs["sf_w"]) * f(inputs["mask"])
    sb1 = wsf @ b1 + f(inputs["sf_b"])
    wsf = wsf * g1[None, :]

    # qkv with LN2 affine folded; q/k pre-scaled by D**-0.25 each
    qsc = float(D) ** -0.25
    wq = f(inputs["wq"]); bq = (wq @ b2 + f(inputs["bq"])) * qsc
    wq = wq * g2[None, :] * qsc
    wk = f(inputs["wk"]); bk = (wk @ b2 + f(inputs["bk"])) * qsc
    wk = wk * g2[None, :] * qsc
    wv = f(inputs["wv"]); bv = wv @ b2 + f(inputs["bv"])
    wv = wv * g2[None, :]

    # ffn with LN3 affine folded
    w1 = f(inputs["ff1_w"]); b1f = w1 @ b3 + f(inputs["ff1_b"])
    w1 = w1 * g3[None, :]

    # column shuffle for head-parity A2A: s = par*512 + j*64 + c  <-  128j+64par+c
    perm = np.empty(H, np.int64)
    for par in range(2):
        for j in range(NC):
            s0 = par * 512 + j * 64
            perm[s0:s0 + 64] = 128 * j + 64 * par + np.arange(64)

    sh = {
        "sfwT": f16(wsf.T),
        "wqT": f16(wq.T[:, perm]),
        "wkT": f16(wk.T[:, perm]),
        "wvT": f16(wv.T[:, perm]),
        "woT": f16(f(inputs["wo"]).T),
        "w1T": f16(w1.T),
        "w2T": f16(f(inputs["ff2_w"]).T),
        "brow": np.ascontiguousarray(np.stack(
            [sb1, bv[perm], f(inputs["bo"]), f(inputs["ff2_b"])]).astype(np.float16)),
        "bcol": np.ascontiguousarray(
            np.stack([bk[perm], bq[perm]]).reshape(2 * KT, 128).T.copy().astype(np.float32)),
        "ff1b_col": np.ascontiguousarray(b1f.reshape(NFT, 128).T.copy().astype(np.float32)),
    }
    # diag masks: tri[i][p, c] = 1 if 128*i + p <= c else 0 (c in 0..512)
    tri = np.zeros((4, 128, 512), np.float32)
    for i in range(4):
        p = np.arange(128)[:, None] + 128 * i
        c = np.arange(512)[None, :]
        tri[i] = (p <= c).astype(np.float32)
    sh["tri"] = tri
    return sh


def kernel(**inputs) -> np.ndarray:
    from concourse.bass_utils import run_bass_kernel_spmd

    if "nc" not in _CACHE:
        _CACHE["nc"] = _build()
    nc = _CACHE["nc"]

    sh = _prep_shared(inputs)
    x = np.ascontiguousarray(np.asarray(inputs["x"], np.float32)).reshape(B * S, H)
    in_maps = []
    for c in range(NC):
        m = dict(sh)
        m["x_c"] = np.ascontiguousarray(x[c * T:(c + 1) * T])
        in_maps.append(m)

    res = run_bass_kernel_spmd(nc, in_maps, core_ids=list(range(NC)))
    out = np.concatenate([res.results[c]["out_c"] for c in range(NC)], axis=0)
    return out.reshape(B, S, H).astype(np.float32)


# revision 51
# speedup vs baseline: 1.1555x; 1.0444x over previous
# Trainium2 Bass kernel for nn_BDHBlock (dense transformer block).
#
# Strategy (8 NeuronCores, one shared SPMD program):
#   - Token-parallel for token-local stages: core c owns flat tokens
#     [512c, 512c+512). LayerNorm affine (g,b) is folded into the following
#     weights on the host, so on-chip LN is a pure normalize. All weights are
#     uploaded fp16 (pre-transposed, pre-shuffled), halving HBM traffic.
#   - Attention is head-parallel: core c owns global heads {2c, 2c+1} over the
#     full 4096-token sequence. q/k/v are exchanged with two AllToAlls split
#     by head parity (lo/hi) so the second exchange and the ctx return ride
#     under attention compute. Attention uses 512-wide q blocks with exact
#     causal masking; score->relu->ctx stages are decoupled across psum banks.
#   - fp16 matmuls everywhere (psum accumulate fp32); bias rows are seeded
#     into psum before accumulation chains to keep vector-engine work low.
import numpy as np

import concourse.bass as bass
import concourse.mybir as mybir
import concourse.tile as tile
from concourse import bacc
from concourse.masks import make_identity

B, S, H, NH = 2, 2048, 1024, 16
D = H // NH            # 64
FF = 4 * H             # 4096
NC = 8                 # cores
T = B * S // NC        # 512 tokens per core
TT = T // 128          # 4 token tiles
KT = H // 128          # 8 feature tiles
NFT = FF // 128        # 32 ffn tiles
F32, F16 = mybir.dt.float32, mybir.dt.float16
ADD, SUB, MUL, MAX = (mybir.AluOpType.add, mybir.AluOpType.subtract,
                      mybir.AluOpType.mult, mybir.AluOpType.max)
AF = mybir.ActivationFunctionType
RG = [list(range(NC))]
EPS = 1e-5

_CACHE = {}


def _build():
    nc = bacc.Bacc("TRN2", target_bir_lowering=False, debug=False,
                   num_devices=NC)

    # ---------------- I/O ----------------
    def inp(name, shape, dtype=F32):
        return nc.dram_tensor(name, list(shape), dtype, kind="ExternalInput")

    x_io = inp("x_c", (T, H))
    sfwT_io = inp("sfwT", (H, H), F16)
    wqT_io = inp("wqT", (H, H), F16)     # col-shuffled (parity, dest, d)
    wkT_io = inp("wkT", (H, H), F16)     # col-shuffled
    wvT_io = inp("wvT", (H, H), F16)     # col-shuffled
    woT_io = inp("woT", (H, H), F16)
    w1T_io = inp("w1T", (H, FF), F16)
    w2T_io = inp("w2T", (FF, H), F16)
    brow_io = inp("brow", (4, H), F16)   # rows: sb1, bv_shuf, bo, ff2_b
    bcol_io = inp("bcol", (128, 2 * KT))  # [p, nt]: bq_shuf, bk_shuf cols
    ff1b_col_io = inp("ff1b_col", (128, NFT))
    tri_io = inp("tri", (4, 128, 512))   # causal diag masks (f32)
    out_io = nc.dram_tensor("out_c", [T, H], F32, kind="ExternalOutput")

    # internal DRAM for collectives (HBM bounce)
    QSL = 64 * T                          # 32768: one head x 512 tokens
    a2a_in = [nc.dram_tensor(f"a2a{p}_in", [NC, 3, QSL], F16) for p in (0, 1)]
    a2a_out = [nc.dram_tensor(f"a2a{p}_out", [NC, 3, QSL], F16) for p in (0, 1)]
    cc_in = [nc.dram_tensor(f"cc{p}_in", [NC, QSL], F16) for p in (0, 1)]
    cc_out = [nc.dram_tensor(f"cc{p}_out", [NC, QSL], F16) for p in (0, 1)]
    warm_in = nc.dram_tensor("warm_in", [NC, 64], F16)
    warm_out = nc.dram_tensor("warm_out", [NC, 64], F16)

    from contextlib import ExitStack
    with tile.TileContext(nc) as tc, ExitStack() as es:
        # ---------------- pools ----------------
        const = es.enter_context(tc.tile_pool(name="const", bufs=1))
        persist = es.enter_context(tc.tile_pool(name="persist", bufs=1))
        wpool = es.enter_context(tc.tile_pool(name="wpool", bufs=6))
        att_pool = es.enter_context(tc.tile_pool(name="attp", bufs=18))
        sc_pool = es.enter_context(tc.tile_pool(name="scratch", bufs=2))
        small = es.enter_context(tc.tile_pool(name="small", bufs=4))
        rr_pool = es.enter_context(tc.tile_pool(name="rrp", bufs=2))
        pacc = es.enter_context(tc.tile_pool(name="pacc", bufs=1, space="PSUM"))
        pcx = es.enter_context(tc.tile_pool(name="pcx", bufs=2, space="PSUM"))
        pmix = es.enter_context(tc.tile_pool(name="pmix", bufs=2, space="PSUM"))

        ident16 = const.tile([128, 128], F16)
        make_identity(nc, ident16)
        # tiny warm-up AllToAll: absorbs launch skew + ring cold-start before
        # the first real exchange
        nc.gpsimd.dma_start(out=warm_in.ap(), in_=ident16[0:NC, 0:64])
        nc.gpsimd.collective_compute(
            "AllToAll", mybir.AluOpType.bypass, replica_groups=RG,
            ins=[warm_in.ap().opt()], outs=[warm_out.ap().opt()])
        tri = const.tile([128, 4, 512], F32)
        nc.sync.dma_start(out=tri[:], in_=tri_io.ap().rearrange("a p q -> p a q"))
        bcol = const.tile([128, 2 * KT], F32)
        nc.sync.dma_start(out=bcol[:], in_=bcol_io.ap())
        ff1b_col = const.tile([128, NFT], F32)
        nc.sync.dma_start(out=ff1b_col[:], in_=ff1b_col_io.ap())
        eps_col = const.tile([128, 1], F32)
        nc.vector.memset(eps_col[:], EPS)
        ones64 = const.tile([1, 64], F32)
        nc.vector.memset(ones64[:], 1.0)
        brow = const.tile([128, 4, H], F16)
        nc.sync.dma_start(
            out=brow[:], in_=brow_io.ap().unsqueeze(0).partition_broadcast(128))

        # residual stream, token-major [128, tt, H] (per-tile loads so LN1 can
        # start on tile 0 while the rest stream in)
        x_sb = persist.tile([128, TT, H], F32)
        for tt in range(TT):
            nc.sync.dma_start(
                out=x_sb[:, tt, :],
                in_=x_io.ap().rearrange("(tt p) h -> p tt h", p=128)[:, tt, :])

        lnT = persist.tile([128, KT, T], F16)       # LN output, feature-major
        kq_local = persist.tile([128, 2, 2, 4, T], F16, tag="bigA")  # [p,(k|q),par,m,t]
        vb_local = persist.tile([128, 2, NC, TT * 64], F16, tag="bigB")  # [p,par,j,(tt c)]
        qa = persist.tile([64, 2, B * S], F16, tag="bigC")  # [d, head, flat tok]
        ka = persist.tile([64, 2, B * S], F16, tag="bigD")
        va = persist.tile([128, 32, 2, D + 1], F16)  # [tokpart, fkt, head, d+1]
        va64 = persist.tile([128, 2, 32, D], F16)    # contiguous A2A landing pad
        ctxT = persist.tile([64, 2, B * S], F16)
        # slot-sharing: kq/vb are dead once the qkv A2As are staged; q/k are
        # dead once attention finishes -> reuse their SBUF for ctx and ffn h.
        ctxo_p = [persist.tile([64, KT, T], F16, tag="bigA", name="ctxo0"),
                  persist.tile([64, KT, T], F16, tag="bigB", name="ctxo1")]
        h_parts = [persist.tile([128, NFT // 2, T], F16, tag="bigC", name="h_lo"),
                   persist.tile([128, NFT // 2, T], F16, tag="bigD", name="h_hi")]
        stat4 = persist.tile([128, 6, TT], F32)     # sums/sq/mu/ex2/var/rstd
        junk = persist.tile([128, H], F32)

        nc.vector.memset(va[:], 1.0)                # bakes the ones column

        # ---------------- psum helpers ----------------
        _round = [0]

        def acc_tiles():
            r = _round[0]; _round[0] += 1
            if r % 2 == 0:
                return [pacc.tile([128, 512], F32, tag=f"acc{t}", name=f"acc{t}")
                        for t in range(4)]
            return [pcx.tile([128, 512], F32, tag="cx", name=f"apc{t}") for t in range(2)] + \
                   [pmix.tile([128, 512], F32, tag="pm", name=f"apm{t}") for t in range(2)]

        _eng = [0]

        def alt():
            _eng[0] ^= 1
            return nc.vector if _eng[0] else nc.scalar

        def copy_alt(out, in_):
            _eng[0] ^= 1
            if _eng[0]:
                nc.vector.tensor_copy(out, in_)
            else:
                nc.scalar.copy(out, in_)

        # ---------------- layernorm (pure normalize) -> lnT ----------------
        def layer_norm_t():
            for tt in range(TT):
                xt = x_sb[:, tt, :]
                sm = stat4[:, 0, tt:tt + 1]
                sq = stat4[:, 1, tt:tt + 1]
                mu = stat4[:, 2, tt:tt + 1]
                var = stat4[:, 4, tt:tt + 1]
                rstd = stat4[:, 5, tt:tt + 1]
                nc.vector.reduce_sum(sm, xt, axis=mybir.AxisListType.X)
                nc.scalar.activation(junk[:], xt, AF.Square, accum_out=sq)
                nc.vector.tensor_scalar_mul(mu, sm, 1.0 / H)
                nc.vector.tensor_scalar(var, mu, mu, -1.0, op0=MUL, op1=MUL)
                nc.vector.scalar_tensor_tensor(var, sq, 1.0 / H, var, op0=MUL, op1=ADD)
                nc.scalar.activation(rstd, var, AF.Sqrt, bias=eps_col[:])
                nc.vector.reciprocal(rstd, rstd)
                z16 = sc_pool.tile([128, H], F16, tag="z16")
                nc.vector.tensor_scalar(z16[:], xt, mu, rstd, op0=SUB, op1=MUL)
                for half in range(2):
                    pt = pmix.tile([128, 512], F16, tag="pm", name="pt")
                    for jj in range(4):
                        nc.tensor.transpose(pt[:, bass.ts(jj, 128)],
                                            z16[:, bass.ts(4 * half + jj, 128)], ident16[:])
                    copy_alt(lnT[:, 4 * half:4 * half + 4, bass.ts(tt, 128)],
                             pt[:].rearrange("p (a q) -> p a q", a=4))

        # ---------------- generic token-major linear + residual ----------------
        def linear_residual(wT_io_, brow_idx, src):
            """x += src.T @ w + bias ; src is lnT-style [128, KT, T] fp16."""
            psg = []
            for nch in range(2):
                ps = acc_tiles()
                psg.append(ps)
                for tt in range(TT):
                    nc.scalar.activation(ps[tt][:], brow[:, brow_idx, bass.ts(nch, 512)],
                                         AF.Copy)
                for kt in range(KT):
                    wt = wpool.tile([128, 512], F16, tag="w")
                    nc.sync.dma_start(out=wt[:], in_=wT_io_.ap()[bass.ts(kt, 128), bass.ts(nch, 512)])
                    for tt in range(TT):
                        nc.tensor.matmul(ps[tt][:], src[:, kt, bass.ts(tt, 128)],
                                         wt[:], start=False, stop=(kt == KT - 1),
                                         skip_group_check=True)
            for tt in range(TT):          # token-major so the next LN starts early
                for nch in range(2):
                    nc.vector.tensor_add(x_sb[:, tt, bass.ts(nch, 512)],
                                         x_sb[:, tt, bass.ts(nch, 512)], psg[nch][tt][:])

        # =====================================================================
        # Stage 1: x += LN1(x) @ (sf_w*mask*g1).T + (sfw@b1 + sf_b)
        # =====================================================================
        layer_norm_t()
        linear_residual(sfwT_io, 0, lnT)

        # =====================================================================
        # Stage 2: LN2 + QKV (parity-split) + A2A
        # =====================================================================
        layer_norm_t()

        def kq_proj(wio, which, par):
            """feature-major: kq_local[:, which, par, m, :] = w_chunk.T @ lnT + bias."""
            ps = acc_tiles()
            for kt in range(KT):
                wt = wpool.tile([128, 512], F16, tag="w")
                nc.sync.dma_start(out=wt[:], in_=wio.ap()[bass.ts(kt, 128), bass.ts(par, 512)])
                for m in range(4):
                    nc.tensor.matmul(ps[m][:], wt[:, bass.ts(m, 128)],
                                     lnT[:, kt, :],
                                     start=(kt == 0), stop=(kt == KT - 1))
            for m in range(4):
                nt = par * 4 + m
                bc = bcol[:, which * KT + nt: which * KT + nt + 1]
                e = alt()
                if e is nc.scalar:
                    nc.scalar.activation(kq_local[:, which, par, m, :], ps[m][:],
                                         AF.Identity, bias=bc)
                else:
                    nc.vector.tensor_scalar_add(kq_local[:, which, par, m, :], ps[m][:], bc)

        def v_proj(par):
            ps = acc_tiles()
            for tt in range(TT):
                nc.scalar.activation(ps[tt][:], brow[:, 1, bass.ts(par, 512)], AF.Copy)
            for kt in range(KT):
                wt = wpool.tile([128, 512], F16, tag="w")
                nc.sync.dma_start(out=wt[:], in_=wvT_io.ap()[bass.ts(kt, 128), bass.ts(par, 512)])
                for tt in range(TT):
                    nc.tensor.matmul(ps[tt][:], lnT[:, kt, bass.ts(tt, 128)],
                                     wt[:], start=False, stop=(kt == KT - 1),
                                     skip_group_check=True)
            for tt in range(TT):
                copy_alt(vb_local[:, par, :, bass.ds(64 * tt, 64)],
                         ps[tt][:].rearrange("p (j c) -> p j c", j=NC))

        def stage_qkv(par):
            # consolidated sends: slot j = 2m + half(partition)
            kq_view = a2a_in[par].ap().rearrange(
                "(m h) r (d t) -> h d r m t", h=2, d=64)
            for which in (0, 1):
                for half in (0, 1):
                    nc.gpsimd.dma_start(
                        out=kq_view[half, :, which, :, :],
                        in_=kq_local[bass.ts(half, 64), which, par, :, :])
            v_view = a2a_in[par].ap().rearrange(
                "j r (p x) -> p r j x", p=128)
            nc.gpsimd.dma_start(out=v_view[:, 2, :, :],
                                in_=vb_local[:, par, :, :])
            nc.gpsimd.collective_compute(
                "AllToAll", mybir.AluOpType.bypass, replica_groups=RG,
                ins=[a2a_in[par].ap().opt()], outs=[a2a_out[par].ap().opt()])

        def unstage_qkv(par):
            kq_view = a2a_out[par].ap().rearrange(
                "j r (d t) -> d r j t", d=64)
            nc.gpsimd.dma_start(
                out=ka[:, par, :].rearrange("p (j t) -> p j t", j=NC),
                in_=kq_view[:, 0, :, :])
            nc.gpsimd.dma_start(
                out=qa[:, par, :].rearrange("p (j t) -> p j t", j=NC),
                in_=kq_view[:, 1, :, :])
            v_view = a2a_out[par].ap().rearrange(
                "j r (p x) -> p r j x", p=128)
            nc.gpsimd.dma_start(
                out=va64[:, par, :, :].rearrange("p (j tt) c -> p j (tt c)", j=NC),
                in_=v_view[:, 2, :, :])
            nc.gpsimd.tensor_copy(va[:, :, par, 0:D], va64[:, par, :, :])

        # parity A compute + send
        kq_proj(wkT_io, 0, 0)
        kq_proj(wqT_io, 1, 0)
        v_proj(0)
        stage_qkv(0)
        unstage_qkv(0)
        # parity B compute + send (overlaps A2A-A)
        kq_proj(wkT_io, 0, 1)
        kq_proj(wqT_io, 1, 1)
        v_proj(1)
        stage_qkv(1)
        unstage_qkv(1)

        # =====================================================================
        # Attention: heads {2c (par 0), 2c+1 (par 1)}, exact causal, relu-norm
        # =====================================================================
        def attention_head(h):
            for b in range(B):
                base = b * S
                for qp in range(S // 512):
                    nkt = 4 * qp + 4
                    cx = pcx.tile([65, 512], F32, tag="cx", name="cx")
                    atts = {}
                    for kt in range(nkt):
                        sc = pacc.tile([128, 512], F32, tag=f"acc{kt % 4}", name="sc")
                        att = att_pool.tile([128, 512], F16, tag="att")
                        nc.tensor.matmul(sc[:], ka[:, h, bass.ds(base + kt * 128, 128)],
                                         qa[:, h, bass.ds(base + qp * 512, 512)],
                                         start=True, stop=True)
                        if kt < 4 * qp:
                            if kt % 2 == 0:
                                nc.scalar.activation(att[:], sc[:], AF.Relu)
                            else:
                                nc.vector.tensor_relu(att[:], sc[:])
                        else:
                            nc.vector.scalar_tensor_tensor(
                                att[:], sc[:], 0.0, tri[:, kt - 4 * qp, :],
                                op0=MAX, op1=MUL)
                        atts[kt] = att
                    for kt in range(nkt):
                        nc.tensor.matmul(cx[:], va[:, b * 16 + kt, h, :],
                                         atts.pop(kt)[:],
                                         start=(kt == 0), stop=(kt == nkt - 1))
                    rs = rr_pool.tile([1, 512], F32, tag="rs")
                    rb = rr_pool.tile([64, 512], F32, tag="rb")
                    nc.vector.tensor_scalar_add(rs[:], cx[64:65, :], 1e-9)
                    nc.vector.reciprocal(rs[:], rs[:])
                    rbp = pmix.tile([64, 512], F32, tag="pm", name="rbp")
                    nc.tensor.matmul(rbp[:], ones64[:1, :], rs[:1, :], start=True, stop=True)
                    nc.vector.tensor_copy(rb[:], rbp[:])
                    nc.vector.tensor_tensor(
                        ctxT[:, h, bass.ds(base + qp * 512, 512)],
                        cx[0:64, :], rb[:], MUL)

        def stage_ctx(par):
            nc.gpsimd.dma_start(
                out=cc_in[par].ap().rearrange("j (d t) -> d j t", d=64),
                in_=ctxT[:, par, :].rearrange("p (j t) -> p j t", j=NC))
            nc.gpsimd.collective_compute(
                "AllToAll", mybir.AluOpType.bypass, replica_groups=RG,
                ins=[cc_in[par].ap().opt()], outs=[cc_out[par].ap().opt()])

        def unstage_ctx(par):
            nc.gpsimd.dma_start(
                out=ctxo_p[par][:],
                in_=cc_out[par].ap().rearrange("j (d t) -> d j t", d=64))

        attention_head(0)
        stage_ctx(0)
        attention_head(1)
        unstage_ctx(0)
        stage_ctx(1)
        unstage_ctx(1)

        # =====================================================================
        # out-proj: x += ctx @ wo.T + bo, parity-split so the par-0 half of the
        # contraction runs while the parity-1 ctx AllToAll is still in flight
        # =====================================================================
        psg = [acc_tiles(), acc_tiles()]
        for nch in range(2):
            for tt in range(TT):
                nc.scalar.activation(psg[nch][tt][:], brow[:, 2, bass.ts(nch, 512)],
                                     AF.Copy)
        for par in range(2):
            for nch in range(2):
                for kt in range(KT):
                    wt = wpool.tile([64, 512], F16, tag="wh")
                    nc.sync.dma_start(
                        out=wt[:],
                        in_=woT_io.ap()[bass.ds(128 * kt + 64 * par, 64), bass.ts(nch, 512)])
                    for tt in range(TT):
                        nc.tensor.matmul(psg[nch][tt][:],
                                         ctxo_p[par][:, kt, bass.ts(tt, 128)],
                                         wt[:], start=False,
                                         stop=(par == 1 and kt == KT - 1),
                                         skip_group_check=True)
        for tt in range(TT):
            for nch in range(2):
                nc.vector.tensor_add(x_sb[:, tt, bass.ts(nch, 512)],
                                     x_sb[:, tt, bass.ts(nch, 512)], psg[nch][tt][:])

        # =====================================================================
        # FFN: x += relu(LN3(x) @ w1'.T + b1f) @ w2.T + ff2_b
        # =====================================================================
        layer_norm_t()
        for nh in range(NFT // 4):
            ps = acc_tiles()
            for kt in range(KT):
                wt = wpool.tile([128, 512], F16, tag="w")
                nc.sync.dma_start(out=wt[:], in_=w1T_io.ap()[bass.ts(kt, 128), bass.ts(nh, 512)])
                for m in range(4):
                    nc.tensor.matmul(ps[m][:], wt[:, bass.ts(m, 128)],
                                     lnT[:, kt, :],
                                     start=(kt == 0), stop=(kt == KT - 1))
            for m in range(4):
                nt = nh * 4 + m
                hd = h_parts[nt // (NFT // 2)][:, nt % (NFT // 2), :]
                if m % 2 == 0:
                    nc.scalar.activation(hd, ps[m][:], AF.Relu,
                                         bias=ff1b_col[:, nt:nt + 1])
                else:
                    nc.vector.tensor_scalar(hd, ps[m][:],
                                            ff1b_col[:, nt:nt + 1], 0.0,
                                            op0=ADD, op1=MAX)
        for nch in range(2):
            ps = acc_tiles()
            for tt in range(TT):
                nc.scalar.activation(ps[tt][:], brow[:, 3, bass.ts(nch, 512)], AF.Copy)
            for kt in range(NFT):
                wt = wpool.tile([128, 512], F16, tag="w")
                nc.sync.dma_start(out=wt[:], in_=w2T_io.ap()[bass.ts(kt, 128), bass.ts(nch, 512)])
                hsrc = h_parts[kt // (NFT // 2)]
                for tt in range(TT):
                    nc.tensor.matmul(ps[tt][:], hsrc[:, kt % (NFT // 2), bass.ts(tt, 128)],
                                     wt[:], start=False, stop=(kt == NFT - 1),
                                     skip_group_check=True)
            for tt in range(TT):
                nc.vector.tensor_add(x_sb[:, tt, bass.ts(nch, 512)],
                                     x_sb[:, tt, bass.ts(nch, 512)], ps[tt][:])
            # stream this half of the output while the other half finishes
            nc.sync.dma_start(
                out=out_io.ap().rearrange("(tt p) h -> p tt h", p=128)[:, :, bass.ts(nch, 512)],
                in_=x_sb[:, :, bass.ts(nch, 512)])

    nc.compile()
    return nc


def _prep_shared(inputs):
    f = lambda a: np.asarray(a, np.float32)
    f16 = lambda a: np.ascontiguousarray(np.asarray(a, np.float16))
    g1, b1 = f(inputs["g1"]), f(inputs["b1"])
    g2, b2 = f(inputs["g2"]), f(inputs["b2"])
    g3, b3 = f(inputs["g3"]), f(inputs["b3"])

    # stage-1 sparse linear with LN1 affine folded in
    wsf = f(inputs["sf_w"]) * f(inputs["mask"])
    sb1 = wsf @ b1 + f(inputs["sf_b"])
    wsf = wsf * g1[None, :]

    # qkv with LN2 affine folded; q/k pre-scaled by D**-0.25 each
    qsc = float(D) ** -0.25
    wq = f(inputs["wq"]); bq = (wq @ b2 + f(inputs["bq"])) * qsc
    wq = wq * g2[None, :] * qsc
    wk = f(inputs["wk"]); bk = (wk @ b2 + f(inputs["bk"])) * qsc
    wk = wk * g2[None, :] * qsc
    wv = f(inputs["wv"]); bv = wv @ b2 + f(inputs["bv"])
    wv = wv * g2[None, :]

    # ffn with LN3 affine folded
    w1 = f(inputs["ff1_w"]); b1f = w1 @ b3 + f(inputs["ff1_b"])
    w1 = w1 * g3[None, :]

    # column shuffle for head-parity A2A: s = par*512 + j*64 + c  <-  128j+64par+c
    perm = np.empty(H, np.int64)
    for par in range(2):
        for j in range(NC):
            s0 = par * 512 + j * 64
            perm[s0:s0 + 64] = 128 * j + 64 * par + np.arange(64)

    sh = {
        "sfwT": f16(wsf.T),
        "wqT": f16(wq.T[:, perm]),
        "wkT": f16(wk.T[:, perm]),
        "wvT": f16(wv.T[:, perm]),
        "woT": f16(f(inputs["wo"]).T),
        "w1T": f16(w1.T),
        "w2T": f16(f(inputs["ff2_w"]).T),
        "brow": np.ascontiguousarray(np.stack(
            [sb1, bv[perm], f(inputs["bo"]), f(inputs["ff2_b"])]).astype(np.float16)),
        "bcol": np.ascontiguousarray(
            np.stack([bk[perm], bq[perm]]).reshape(2 * KT, 128).T.copy().astype(np.float32)),
        "ff1b_col": np.ascontiguousarray(b1f.reshape(NFT, 128).T.copy().astype(np.float32)),
    }
    # diag masks: tri[i][p, c] = 1 if 128*i + p <= c else 0 (c in 0..512)
    tri = np.zeros((4, 128, 512), np.float32)
    for i in range(4):
        p = np.arange(128)[:, None] + 128 * i
        c = np.arange(512)[None, :]
        tri[i] = (p <= c).astype(np.float32)
    sh["tri"] = tri
    return sh


def kernel(**inputs) -> np.ndarray:
    from concourse.bass_utils import run_bass_kernel_spmd

    if "nc" not in _CACHE:
        _CACHE["nc"] = _build()
    nc = _CACHE["nc"]

    sh = _prep_shared(inputs)
    x = np.ascontiguousarray(np.asarray(inputs["x"], np.float32)).reshape(B * S, H)
    in_maps = []
    for c in range(NC):
        m = dict(sh)
        m["x_c"] = np.ascontiguousarray(x[c * T:(c + 1) * T])
        in_maps.append(m)

    res = run_bass_kernel_spmd(nc, in_maps, core_ids=list(range(NC)))
    out = np.concatenate([res.results[c]["out_c"] for c in range(NC)], axis=0)
    return out.reshape(B, S, H).astype(np.float32)


# revision 53
# speedup vs baseline: 1.1737x; 1.0157x over previous
# Trainium2 Bass kernel for nn_BDHBlock (dense transformer block).
#
# Strategy (8 NeuronCores, one shared SPMD program):
#   - Token-parallel for token-local stages: core c owns flat tokens
#     [512c, 512c+512). LayerNorm affine (g,b) is folded into the following
#     weights on the host, so on-chip LN is a pure normalize. All weights are
#     uploaded fp16 (pre-transposed, pre-shuffled), halving HBM traffic.
#   - Attention is head-parallel: core c owns global heads {2c, 2c+1} over the
#     full 4096-token sequence. q/k/v are exchanged with two AllToAlls split
#     by head parity (lo/hi) so the second exchange and the ctx return ride
#     under attention compute. Attention uses 512-wide q blocks with exact
#     causal masking; score->relu->ctx stages are decoupled across psum banks.
#   - fp16 matmuls everywhere (psum accumulate fp32); bias rows are seeded
#     into psum before accumulation chains to keep vector-engine work low.
import numpy as np

import concourse.bass as bass
import concourse.mybir as mybir
import concourse.tile as tile
from concourse import bacc
from concourse.masks import make_identity

B, S, H, NH = 2, 2048, 1024, 16
D = H // NH            # 64
FF = 4 * H             # 4096
NC = 8                 # cores
T = B * S // NC        # 512 tokens per core
TT = T // 128          # 4 token tiles
KT = H // 128          # 8 feature tiles
NFT = FF // 128        # 32 ffn tiles
F32, F16 = mybir.dt.float32, mybir.dt.float16
ADD, SUB, MUL, MAX = (mybir.AluOpType.add, mybir.AluOpType.subtract,
                      mybir.AluOpType.mult, mybir.AluOpType.max)
AF = mybir.ActivationFunctionType
RG = [list(range(NC))]
EPS = 1e-5

_CACHE = {}


def _build():
    nc = bacc.Bacc("TRN2", target_bir_lowering=False, debug=False,
                   num_devices=NC)

    # ---------------- I/O ----------------
    def inp(name, shape, dtype=F32):
        return nc.dram_tensor(name, list(shape), dtype, kind="ExternalInput")

    x_io = inp("x_c", (T, H))
    sfwT_io = inp("sfwT", (H, H), F16)
    wqT_io = inp("wqT", (H, H), F16)     # col-shuffled (parity, dest, d)
    wkT_io = inp("wkT", (H, H), F16)     # col-shuffled
    wvT_io = inp("wvT", (H, H), F16)     # col-shuffled
    woT_io = inp("woT", (H, H), F16)
    w1T_io = inp("w1T", (H, FF), F16)
    w2T_io = inp("w2T", (FF, H), F16)
    brow_io = inp("brow", (4, H), F16)   # rows: sb1, bv_shuf, bo, ff2_b
    bcol_io = inp("bcol", (128, 2 * KT))  # [p, nt]: bq_shuf, bk_shuf cols
    ff1b_col_io = inp("ff1b_col", (128, NFT))
    tri_io = inp("tri", (4, 128, 512))   # causal diag masks (f32)
    out_io = nc.dram_tensor("out_c", [T, H], F32, kind="ExternalOutput")

    # internal DRAM for collectives (HBM bounce)
    QSL = 64 * T                          # 32768: one head x 512 tokens
    a2a_in = [nc.dram_tensor(f"a2a{p}_in", [NC, 3, QSL], F16) for p in (0, 1)]
    a2a_out = [nc.dram_tensor(f"a2a{p}_out", [NC, 3, QSL], F16) for p in (0, 1)]
    cc_in = [nc.dram_tensor(f"cc{p}_in", [NC, QSL], F16) for p in (0, 1)]
    cc_out = [nc.dram_tensor(f"cc{p}_out", [NC, QSL], F16) for p in (0, 1)]
    warm_in = nc.dram_tensor("warm_in", [NC, 64], F16)
    warm_out = nc.dram_tensor("warm_out", [NC, 64], F16)

    from contextlib import ExitStack
    with tile.TileContext(nc) as tc, ExitStack() as es:
        # ---------------- pools ----------------
        const = es.enter_context(tc.tile_pool(name="const", bufs=1))
        persist = es.enter_context(tc.tile_pool(name="persist", bufs=1))
        wpool = es.enter_context(tc.tile_pool(name="wpool", bufs=6))
        att_pool = es.enter_context(tc.tile_pool(name="attp", bufs=18))
        sc_pool = es.enter_context(tc.tile_pool(name="scratch", bufs=2))
        small = es.enter_context(tc.tile_pool(name="small", bufs=4))
        rr_pool = es.enter_context(tc.tile_pool(name="rrp", bufs=2))
        pacc = es.enter_context(tc.tile_pool(name="pacc", bufs=1, space="PSUM"))
        pcx = es.enter_context(tc.tile_pool(name="pcx", bufs=2, space="PSUM"))
        pmix = es.enter_context(tc.tile_pool(name="pmix", bufs=2, space="PSUM"))

        ident16 = const.tile([128, 128], F16)
        make_identity(nc, ident16)
        # tiny warm-up AllToAll: absorbs launch skew + ring cold-start before
        # the first real exchange
        nc.gpsimd.dma_start(out=warm_in.ap(), in_=ident16[0:NC, 0:64])
        nc.gpsimd.collective_compute(
            "AllToAll", mybir.AluOpType.bypass, replica_groups=RG,
            ins=[warm_in.ap().opt()], outs=[warm_out.ap().opt()])
        tri = const.tile([128, 4, 512], F32)
        nc.sync.dma_start(out=tri[:], in_=tri_io.ap().rearrange("a p q -> p a q"))
        bcol = const.tile([128, 2 * KT], F32)
        nc.sync.dma_start(out=bcol[:], in_=bcol_io.ap())
        ff1b_col = const.tile([128, NFT], F32)
        nc.sync.dma_start(out=ff1b_col[:], in_=ff1b_col_io.ap())
        eps_col = const.tile([128, 1], F32)
        nc.vector.memset(eps_col[:], EPS)
        ones64 = const.tile([1, 64], F32)
        nc.vector.memset(ones64[:], 1.0)
        brow = const.tile([128, 4, H], F16)
        nc.sync.dma_start(
            out=brow[:], in_=brow_io.ap().unsqueeze(0).partition_broadcast(128))

        # residual stream, token-major [128, tt, H] (per-tile loads so LN1 can
        # start on tile 0 while the rest stream in)
        x_sb = persist.tile([128, TT, H], F32)
        for tt in range(TT):
            nc.sync.dma_start(
                out=x_sb[:, tt, :],
                in_=x_io.ap().rearrange("(tt p) h -> p tt h", p=128)[:, tt, :])

        lnT = persist.tile([128, KT, T], F16)       # LN output, feature-major
        kq_local = persist.tile([128, 2, 2, 4, T], F16, tag="bigA")  # [p,(k|q),par,m,t]
        vb_local = persist.tile([128, 2, NC, TT * 64], F16, tag="bigB")  # [p,par,j,(tt c)]
        qa = persist.tile([64, 2, B * S], F16, tag="bigC")  # [d, head, flat tok]
        ka = persist.tile([64, 2, B * S], F16, tag="bigD")
        va = persist.tile([128, 32, 2, D + 1], F16)  # [tokpart, fkt, head, d+1]
        va64 = persist.tile([128, 2, 32, D], F16)    # contiguous A2A landing pad
        ctxT = persist.tile([64, 2, B * S], F16)
        # slot-sharing: kq/vb are dead once the qkv A2As are staged; q/k are
        # dead once attention finishes -> reuse their SBUF for ctx and ffn h.
        ctxo_p = [persist.tile([64, KT, T], F16, tag="bigA", name="ctxo0"),
                  persist.tile([64, KT, T], F16, tag="bigB", name="ctxo1")]
        h_parts = [persist.tile([128, NFT // 2, T], F16, tag="bigC", name="h_lo"),
                   persist.tile([128, NFT // 2, T], F16, tag="bigD", name="h_hi")]
        stat4 = persist.tile([128, 6, TT], F32)     # sums/sq/mu/ex2/var/rstd
        junk = persist.tile([128, H], F32)

        nc.vector.memset(va[:], 1.0)                # bakes the ones column

        # ---------------- psum helpers ----------------
        _round = [0]

        def acc_tiles():
            r = _round[0]; _round[0] += 1
            if r % 2 == 0:
                return [pacc.tile([128, 512], F32, tag=f"acc{t}", name=f"acc{t}")
                        for t in range(4)]
            return [pcx.tile([128, 512], F32, tag="cx", name=f"apc{t}") for t in range(2)] + \
                   [pmix.tile([128, 512], F32, tag="pm", name=f"apm{t}") for t in range(2)]

        _eng = [0]

        def alt():
            _eng[0] ^= 1
            return nc.vector if _eng[0] else nc.scalar

        def copy_alt(out, in_):
            _eng[0] ^= 1
            if _eng[0]:
                nc.vector.tensor_copy(out, in_)
            else:
                nc.scalar.copy(out, in_)

        # ---------------- layernorm (pure normalize) -> lnT ----------------
        def layer_norm_t():
            for tt in range(TT):
                xt = x_sb[:, tt, :]
                sm = stat4[:, 0, tt:tt + 1]
                sq = stat4[:, 1, tt:tt + 1]
                mu = stat4[:, 2, tt:tt + 1]
                var = stat4[:, 4, tt:tt + 1]
                rstd = stat4[:, 5, tt:tt + 1]
                nc.vector.reduce_sum(sm, xt, axis=mybir.AxisListType.X)
                nc.scalar.activation(junk[:], xt, AF.Square, accum_out=sq)
                nc.vector.tensor_scalar_mul(mu, sm, 1.0 / H)
                nc.vector.tensor_scalar(var, mu, mu, -1.0, op0=MUL, op1=MUL)
                nc.vector.scalar_tensor_tensor(var, sq, 1.0 / H, var, op0=MUL, op1=ADD)
                nc.scalar.activation(rstd, var, AF.Sqrt, bias=eps_col[:])
                nc.vector.reciprocal(rstd, rstd)
                z16 = sc_pool.tile([128, H], F16, tag="z16")
                nc.vector.tensor_scalar(z16[:], xt, mu, rstd, op0=SUB, op1=MUL)
                for half in range(2):
                    pt = pmix.tile([128, 512], F16, tag="pm", name="pt")
                    for jj in range(4):
                        nc.tensor.transpose(pt[:, bass.ts(jj, 128)],
                                            z16[:, bass.ts(4 * half + jj, 128)], ident16[:])
                    copy_alt(lnT[:, 4 * half:4 * half + 4, bass.ts(tt, 128)],
                             pt[:].rearrange("p (a q) -> p a q", a=4))

        # ---------------- generic token-major linear + residual ----------------
        def linear_residual(wT_io_, brow_idx, src):
            """x += src.T @ w + bias ; src is lnT-style [128, KT, T] fp16."""
            psg = []
            for nch in range(2):
                ps = acc_tiles()
                psg.append(ps)
                for tt in range(TT):
                    nc.scalar.activation(ps[tt][:], brow[:, brow_idx, bass.ts(nch, 512)],
                                         AF.Copy)
                for kt in range(KT):
                    wt = wpool.tile([128, 512], F16, tag="w")
                    nc.sync.dma_start(out=wt[:], in_=wT_io_.ap()[bass.ts(kt, 128), bass.ts(nch, 512)])
                    for tt in range(TT):
                        nc.tensor.matmul(ps[tt][:], src[:, kt, bass.ts(tt, 128)],
                                         wt[:], start=False, stop=(kt == KT - 1),
                                         skip_group_check=True)
            for tt in range(TT):          # token-major so the next LN starts early
                for nch in range(2):
                    nc.vector.tensor_add(x_sb[:, tt, bass.ts(nch, 512)],
                                         x_sb[:, tt, bass.ts(nch, 512)], psg[nch][tt][:])

        # =====================================================================
        # Stage 1: x += LN1(x) @ (sf_w*mask*g1).T + (sfw@b1 + sf_b)
        # =====================================================================
        layer_norm_t()
        linear_residual(sfwT_io, 0, lnT)

        # =====================================================================
        # Stage 2: LN2 + QKV (parity-split) + A2A
        # =====================================================================
        layer_norm_t()

        def kq_proj(wio, which, par):
            """feature-major: kq_local[:, which, par, m, :] = w_chunk.T @ lnT + bias."""
            ps = acc_tiles()
            for kt in range(KT):
                wt = wpool.tile([128, 512], F16, tag="w")
                nc.sync.dma_start(out=wt[:], in_=wio.ap()[bass.ts(kt, 128), bass.ts(par, 512)])
                for m in range(4):
                    nc.tensor.matmul(ps[m][:], wt[:, bass.ts(m, 128)],
                                     lnT[:, kt, :],
                                     start=(kt == 0), stop=(kt == KT - 1))
            for m in range(4):
                nt = par * 4 + m
                bc = bcol[:, which * KT + nt: which * KT + nt + 1]
                e = alt()
                if e is nc.scalar:
                    nc.scalar.activation(kq_local[:, which, par, m, :], ps[m][:],
                                         AF.Identity, bias=bc)
                else:
                    nc.vector.tensor_scalar_add(kq_local[:, which, par, m, :], ps[m][:], bc)

        def v_proj(par):
            ps = acc_tiles()
            for tt in range(TT):
                nc.scalar.activation(ps[tt][:], brow[:, 1, bass.ts(par, 512)], AF.Copy)
            for kt in range(KT):
                wt = wpool.tile([128, 512], F16, tag="w")
                nc.sync.dma_start(out=wt[:], in_=wvT_io.ap()[bass.ts(kt, 128), bass.ts(par, 512)])
                for tt in range(TT):
                    nc.tensor.matmul(ps[tt][:], lnT[:, kt, bass.ts(tt, 128)],
                                     wt[:], start=False, stop=(kt == KT - 1),
                                     skip_group_check=True)
            for tt in range(TT):
                copy_alt(vb_local[:, par, :, bass.ds(64 * tt, 64)],
                         ps[tt][:].rearrange("p (j c) -> p j c", j=NC))

        def send_kq(par, which):
            # consolidated sends: slot j = 2m + half(partition)
            kq_view = a2a_in[par].ap().rearrange(
                "(m h) r (d t) -> h d r m t", h=2, d=64)
            for half in (0, 1):
                nc.gpsimd.dma_start(
                    out=kq_view[half, :, which, :, :],
                    in_=kq_local[bass.ts(half, 64), which, par, :, :])

        def send_v(par):
            v_view = a2a_in[par].ap().rearrange(
                "j r (p x) -> p r j x", p=128)
            nc.gpsimd.dma_start(out=v_view[:, 2, :, :],
                                in_=vb_local[:, par, :, :])

        def trigger_qkv(par):
            nc.gpsimd.collective_compute(
                "AllToAll", mybir.AluOpType.bypass, replica_groups=RG,
                ins=[a2a_in[par].ap().opt()], outs=[a2a_out[par].ap().opt()])

        def unstage_qkv(par):
            kq_view = a2a_out[par].ap().rearrange(
                "j r (d t) -> d r j t", d=64)
            nc.gpsimd.dma_start(
                out=ka[:, par, :].rearrange("p (j t) -> p j t", j=NC),
                in_=kq_view[:, 0, :, :])
            nc.gpsimd.dma_start(
                out=qa[:, par, :].rearrange("p (j t) -> p j t", j=NC),
                in_=kq_view[:, 1, :, :])
            v_view = a2a_out[par].ap().rearrange(
                "j r (p x) -> p r j x", p=128)
            nc.gpsimd.dma_start(
                out=va64[:, par, :, :].rearrange("p (j tt) c -> p j (tt c)", j=NC),
                in_=v_view[:, 2, :, :])
            nc.gpsimd.tensor_copy(va[:, :, par, 0:D], va64[:, par, :, :])

        # parity A compute + send (each tensor's send fires as soon as ready)
        v_proj(0); send_v(0)
        kq_proj(wkT_io, 0, 0); send_kq(0, 0)
        kq_proj(wqT_io, 1, 0); send_kq(0, 1)
        trigger_qkv(0)
        unstage_qkv(0)
        # parity B compute + send (overlaps A2A-A)
        v_proj(1); send_v(1)
        kq_proj(wkT_io, 0, 1); send_kq(1, 0)
        kq_proj(wqT_io, 1, 1); send_kq(1, 1)
        trigger_qkv(1)
        unstage_qkv(1)

        # =====================================================================
        # Attention: heads {2c (par 0), 2c+1 (par 1)}, exact causal, relu-norm
        # =====================================================================
        def attention_head(h):
            for b in range(B):
                base = b * S
                for qp in range(S // 512):
                    nkt = 4 * qp + 4
                    cx = pcx.tile([65, 512], F32, tag="cx", name="cx")
                    atts = {}
                    for kt in range(nkt):
                        sc = pacc.tile([128, 512], F32, tag=f"acc{kt % 4}", name="sc")
                        att = att_pool.tile([128, 512], F16, tag="att")
                        nc.tensor.matmul(sc[:], ka[:, h, bass.ds(base + kt * 128, 128)],
                                         qa[:, h, bass.ds(base + qp * 512, 512)],
                                         start=True, stop=True)
                        if kt < 4 * qp:
                            if kt % 2 == 0:
                                nc.scalar.activation(att[:], sc[:], AF.Relu)
                            else:
                                nc.vector.tensor_relu(att[:], sc[:])
                        else:
                            nc.vector.scalar_tensor_tensor(
                                att[:], sc[:], 0.0, tri[:, kt - 4 * qp, :],
                                op0=MAX, op1=MUL)
                        atts[kt] = att
                    for kt in range(nkt):
                        nc.tensor.matmul(cx[:], va[:, b * 16 + kt, h, :],
                                         atts.pop(kt)[:],
                                         start=(kt == 0), stop=(kt == nkt - 1))
                    rs = rr_pool.tile([1, 512], F32, tag="rs")
                    rb = rr_pool.tile([64, 512], F32, tag="rb")
                    nc.vector.tensor_scalar_add(rs[:], cx[64:65, :], 1e-9)
                    nc.vector.reciprocal(rs[:], rs[:])
                    rbp = pmix.tile([64, 512], F32, tag="pm", name="rbp")
                    nc.tensor.matmul(rbp[:], ones64[:1, :], rs[:1, :], start=True, stop=True)
                    nc.vector.tensor_copy(rb[:], rbp[:])
                    nc.vector.tensor_tensor(
                        ctxT[:, h, bass.ds(base + qp * 512, 512)],
                        cx[0:64, :], rb[:], MUL)

        def stage_ctx(par):
            nc.gpsimd.dma_start(
                out=cc_in[par].ap().rearrange("j (d t) -> d j t", d=64),
                in_=ctxT[:, par, :].rearrange("p (j t) -> p j t", j=NC))
            nc.gpsimd.collective_compute(
                "AllToAll", mybir.AluOpType.bypass, replica_groups=RG,
                ins=[cc_in[par].ap().opt()], outs=[cc_out[par].ap().opt()])

        def unstage_ctx(par):
            nc.gpsimd.dma_start(
                out=ctxo_p[par][:],
                in_=cc_out[par].ap().rearrange("j (d t) -> d j t", d=64))

        attention_head(0)
        stage_ctx(0)
        attention_head(1)
        unstage_ctx(0)
        stage_ctx(1)
        unstage_ctx(1)

        # =====================================================================
        # out-proj: x += ctx @ wo.T + bo, parity-split so the par-0 half of the
        # contraction runs while the parity-1 ctx AllToAll is still in flight
        # =====================================================================
        psg = [acc_tiles(), acc_tiles()]
        for nch in range(2):
            for tt in range(TT):
                nc.scalar.activation(psg[nch][tt][:], brow[:, 2, bass.ts(nch, 512)],
                                     AF.Copy)
        for par in range(2):
            for nch in range(2):
                for kt in range(KT):
                    wt = wpool.tile([64, 512], F16, tag="wh")
                    nc.sync.dma_start(
                        out=wt[:],
                        in_=woT_io.ap()[bass.ds(128 * kt + 64 * par, 64), bass.ts(nch, 512)])
                    for tt in range(TT):
                        nc.tensor.matmul(psg[nch][tt][:],
                                         ctxo_p[par][:, kt, bass.ts(tt, 128)],
                                         wt[:], start=False,
                                         stop=(par == 1 and kt == KT - 1),
                                         skip_group_check=True)
        for tt in range(TT):
            for nch in range(2):
                nc.vector.tensor_add(x_sb[:, tt, bass.ts(nch, 512)],
                                     x_sb[:, tt, bass.ts(nch, 512)], psg[nch][tt][:])

        # =====================================================================
        # FFN: x += relu(LN3(x) @ w1'.T + b1f) @ w2.T + ff2_b
        # =====================================================================
        layer_norm_t()
        for nh in range(NFT // 4):
            ps = acc_tiles()
            for kt in range(KT):
                wt = wpool.tile([128, 512], F16, tag="w")
                nc.sync.dma_start(out=wt[:], in_=w1T_io.ap()[bass.ts(kt, 128), bass.ts(nh, 512)])
                for m in range(4):
                    nc.tensor.matmul(ps[m][:], wt[:, bass.ts(m, 128)],
                                     lnT[:, kt, :],
                                     start=(kt == 0), stop=(kt == KT - 1))
            for m in range(4):
                nt = nh * 4 + m
                hd = h_parts[nt // (NFT // 2)][:, nt % (NFT // 2), :]
                if m % 2 == 0:
                    nc.scalar.activation(hd, ps[m][:], AF.Relu,
                                         bias=ff1b_col[:, nt:nt + 1])
                else:
                    nc.vector.tensor_scalar(hd, ps[m][:],
                                            ff1b_col[:, nt:nt + 1], 0.0,
                                            op0=ADD, op1=MAX)
        for nch in range(2):
            ps = acc_tiles()
            for tt in range(TT):
                nc.scalar.activation(ps[tt][:], brow[:, 3, bass.ts(nch, 512)], AF.Copy)
            for kt in range(NFT):
                wt = wpool.tile([128, 512], F16, tag="w")
                nc.sync.dma_start(out=wt[:], in_=w2T_io.ap()[bass.ts(kt, 128), bass.ts(nch, 512)])
                hsrc = h_parts[kt // (NFT // 2)]
                for tt in range(TT):
                    nc.tensor.matmul(ps[tt][:], hsrc[:, kt % (NFT // 2), bass.ts(tt, 128)],
                                     wt[:], start=False, stop=(kt == NFT - 1),
                                     skip_group_check=True)
            for tt in range(TT):
                nc.vector.tensor_add(x_sb[:, tt, bass.ts(nch, 512)],
                                     x_sb[:, tt, bass.ts(nch, 512)], ps[tt][:])
            # stream this half of the output while the other half finishes
            nc.sync.dma_start(
                out=out_io.ap().rearrange("(tt p) h -> p tt h", p=128)[:, :, bass.ts(nch, 512)],
                in_=x_sb[:, :, bass.ts(nch, 512)])

    nc.compile()
    return nc


def _prep_shared(inputs):
    f = lambda a: np.asarray(a, np.float32)
    f16 = lambda a: np.ascontiguousarray(np.asarray(a, np.float16))
    g1, b1 = f(inputs["g1"]), f(inputs["b1"])
    g2, b2 = f(inputs["g2"]), f(inputs["b2"])
    g3, b3 = f(inputs["g3"]), f(inputs["b3"])

    # stage-1 sparse linear with LN1 affine folded in
    wsf = f(inputs["sf_w"]) * f(inputs["mask"])
    sb1 = wsf @ b1 + f(inputs["sf_b"])
    wsf = wsf * g1[None, :]

    # qkv with LN2 affine folded; q/k pre-scaled by D**-0.25 each
    qsc = float(D) ** -0.25
    wq = f(inputs["wq"]); bq = (wq @ b2 + f(inputs["bq"])) * qsc
    wq = wq * g2[None, :] * qsc
    wk = f(inputs["wk"]); bk = (wk @ b2 + f(inputs["bk"])) * qsc
    wk = wk * g2[None, :] * qsc
    wv = f(inputs["wv"]); bv = wv @ b2 + f(inputs["bv"])
    wv = wv * g2[None, :]

    # ffn with LN3 affine folded
    w1 = f(inputs["ff1_w"]); b1f = w1 @ b3 + f(inputs["ff1_b"])
    w1 = w1 * g3[None, :]

    # column shuffle for head-parity A2A: s = par*512 + j*64 + c  <-  128j+64par+c
    perm = np.empty(H, np.int64)
    for par in range(2):
        for j in range(NC):
            s0 = par * 512 + j * 64
            perm[s0:s0 + 64] = 128 * j + 64 * par + np.arange(64)

    sh = {
        "sfwT": f16(wsf.T),
        "wqT": f16(wq.T[:, perm]),
        "wkT": f16(wk.T[:, perm]),
        "wvT": f16(wv.T[:, perm]),
        "woT": f16(f(inputs["wo"]).T),
        "w1T": f16(w1.T),
        "w2T": f16(f(inputs["ff2_w"]).T),
        "brow": np.ascontiguousarray(np.stack(
            [sb1, bv[perm], f(inputs["bo"]), f(inputs["ff2_b"])]).astype(np.float16)),
        "bcol": np.ascontiguousarray(
            np.stack([bk[perm], bq[perm]]).reshape(2 * KT, 128).T.copy().astype(np.float32)),
        "ff1b_col": np.ascontiguousarray(b1f.reshape(NFT, 128).T.copy().astype(np.float32)),
    }
    # diag masks: tri[i][p, c] = 1 if 128*i + p <= c else 0 (c in 0..512)
    tri = np.zeros((4, 128, 512), np.float32)
    for i in range(4):
        p = np.arange(128)[:, None] + 128 * i
        c = np.arange(512)[None, :]
        tri[i] = (p <= c).astype(np.float32)
    sh["tri"] = tri
    return sh


def kernel(**inputs) -> np.ndarray:
    from concourse.bass_utils import run_bass_kernel_spmd

    if "nc" not in _CACHE:
        _CACHE["nc"] = _build()
    nc = _CACHE["nc"]

    sh = _prep_shared(inputs)
    x = np.ascontiguousarray(np.asarray(inputs["x"], np.float32)).reshape(B * S, H)
    in_maps = []
    for c in range(NC):
        m = dict(sh)
        m["x_c"] = np.ascontiguousarray(x[c * T:(c + 1) * T])
        in_maps.append(m)

    res = run_bass_kernel_spmd(nc, in_maps, core_ids=list(range(NC)))
    out = np.concatenate([res.results[c]["out_c"] for c in range(NC)], axis=0)
    return out.reshape(B, S, H).astype(np.float32)
